# revision 15
# baseline (speedup 1.0000x reference)
"""Trainium2 Bass kernel for a dense transformer block (pre-LN, MHA + GELU MLP).

Sharding: 8 cores = 2 batches x 4 sequence-quarters. Each core recomputes
LN1 + K/V for its full batch (zero cross-core communication), and computes
Q/attention/proj/MLP for its own 512 tokens only.

Device works feature-major ([feature, token]); the host pre-transposes x and
post-transposes the output. LN gains/biases are folded into the following
matmul weights on the host; the qk scale (1/8) is folded into W_q; the v bias
is folded into b_proj.

Numerics: matmul operands are bf16 (fp32 PSUM accumulation); the residual
stream (x, x2, out), layernorm statistics, and softmax denominators stay fp32.
LN-statistic / broadcast matmuls run in fp32r.
"""
import sys

sys.path.insert(0, "/opt/trn_rl_repo")

import numpy as np
import ml_dtypes

import concourse.bass as bass  # noqa: F401
import concourse.tile as tile
from concourse import bacc, mybir, bass_utils

F32 = mybir.dt.float32
F32R = mybir.dt.float32r
BF16 = mybir.dt.bfloat16
AF = mybir.ActivationFunctionType
ALU = mybir.AluOpType

P = 128
D = 768
NH = 12
DH = 64
DFF = 3072
TB = 2048      # tokens per batch
TO = 512       # tokens owned per core
NJ = D // P    # 6 feature tiles
NT = TB // TO  # 4 token tiles per batch
NTK = TB // P  # 16 key tiles
NMLP = DFF // P  # 24
EPS = 1e-6
N_CORES = 8
VW = 66        # 64 v cols + 2 ones cols per head


def R(ap):
    return ap.bitcast(F32R)


def _build():
    nc = bacc.Bacc("TRN2", target_bir_lowering=False, debug=False,
                   num_devices=N_CORES)

    x_fm = nc.dram_tensor("x_fm", [D, TB], BF16, kind="ExternalInput").ap()
    x_own_d = nc.dram_tensor("x_own", [D, TO], F32, kind="ExternalInput").ap()
    wqkv = nc.dram_tensor("wqkv", [D, 3 * D], BF16, kind="ExternalInput").ap()
    bqk = nc.dram_tensor("bqk", [P, 12], F32, kind="ExternalInput").ap()
    wproj = nc.dram_tensor("wproj", [D, D], BF16, kind="ExternalInput").ap()
    bproj = nc.dram_tensor("bproj", [P, NJ], F32, kind="ExternalInput").ap()
    wfc1 = nc.dram_tensor("wfc1", [D, DFF], BF16, kind="ExternalInput").ap()
    bfc1 = nc.dram_tensor("bfc1", [P, NMLP], F32, kind="ExternalInput").ap()
    wfc2 = nc.dram_tensor("wfc2", [DFF, D], BF16, kind="ExternalInput").ap()
    bfc2 = nc.dram_tensor("bfc2", [P, NJ], F32, kind="ExternalInput").ap()
    out_fm = nc.dram_tensor("out_fm", [D, TO], F32, kind="ExternalOutput").ap()

    with nc.allow_low_precision(reason="bf16 matmul operands are intentional"), \
            tile.TileContext(nc) as tc:
        _emit(tc, nc, x_fm, x_own_d, wqkv, bqk, wproj, bproj, wfc1, bfc1,
              wfc2, bfc2, out_fm)
    nc.compile()
    return nc


def _ln_batched(nc, pools, x_tiles_by_nt, xn_out_fn, ones_t, half2, eps2,
                n_nt, bf16_in):
    """LayerNorm over n_nt token tiles of 512, one batched stats chain.

    x_tiles_by_nt[nt][j]: input tiles [128, 512] (bf16 if bf16_in else fp32).
    xn_out_fn(nt, j) -> bf16 dest AP [128, 512].
    """
    tc, stats, sq_pool, ln_ps, bc_ps = pools
    ntot = 512 * n_nt
    sum_sb = stats.tile([2, ntot], F32, tag="sum_sb", name="sum_sb")
    sq_sb = stats.tile([2, ntot], F32, tag="sq_sb", name="sq_sb")
    mk = (lambda ap: ap) if bf16_in else R
    sqdt = BF16 if bf16_in else F32
    for nt in range(n_nt):
        x_tiles = x_tiles_by_nt[nt]
        xsq = []
        for j in range(NJ):
            t = sq_pool.tile([P, 512], sqdt, tag=f"xsq{j}", name="xsqt")
            nc.scalar.activation(out=mk(t), in_=x_tiles[j], func=AF.Square)
            xsq.append(t)
        ps_sum = ln_ps.tile([2, 512], F32, tag="lnsum", name="ps_sum")
        ps_sq = ln_ps.tile([2, 512], F32, tag="lnsq", name="ps_sq")
        for j in range(NJ):
            nc.tensor.matmul(ps_sum[:], lhsT=mk(ones_t), rhs=mk(x_tiles[j]),
                             start=(j == 0), stop=(j == NJ - 1))
        for j in range(NJ):
            nc.tensor.matmul(ps_sq[:], lhsT=mk(ones_t), rhs=mk(xsq[j]),
                             start=(j == 0), stop=(j == NJ - 1))
        sl = slice(nt * 512, (nt + 1) * 512)
        nc.vector.tensor_copy(out=sum_sb[:, sl], in_=ps_sum)
        nc.vector.tensor_copy(out=sq_sb[:, sl], in_=ps_sq)
    # var*D^2 = D*sumsq - sum^2 ; rs = exp(-0.5*ln(varD2/D^2 + eps))
    t1 = stats.tile([2, ntot], F32, tag="t1", name="t1")
    nc.vector.scalar_tensor_tensor(out=t1, in0=sum_sb, scalar=-1.0,
                                   in1=sum_sb, op0=ALU.mult, op1=ALU.mult)
    nc.vector.scalar_tensor_tensor(out=t1, in0=sq_sb, scalar=float(D),
                                   in1=t1, op0=ALU.mult, op1=ALU.add)
    nc.scalar.activation(out=t1, in_=t1, func=AF.Ln, bias=eps2,
                         scale=1.0 / (D * D))
    rs = stats.tile([2, ntot], F32, tag="rs", name="rs")
    nc.scalar.activation(out=R(rs), in_=t1, func=AF.Exp, scale=-0.5)
    cc = stats.tile([2, ntot], F32, tag="cc", name="cc")
    nc.vector.scalar_tensor_tensor(out=R(cc), in0=sum_sb, scalar=-1.0 / D,
                                   in1=rs, op0=ALU.mult, op1=ALU.mult)
    for nt in range(n_nt):
        sl = slice(nt * 512, (nt + 1) * 512)
        ps_a = bc_ps.tile([P, 512], F32, tag="bca", name="ps_a")
        nc.tensor.matmul(ps_a[:], lhsT=R(half2), rhs=R(rs[:, sl]),
                         start=True, stop=True)
        ps_c = bc_ps.tile([P, 512], F32, tag="bcc", name="ps_c")
        nc.tensor.matmul(ps_c[:], lhsT=R(half2), rhs=R(cc[:, sl]),
                         start=True, stop=True)
        for j in range(NJ):
            tmp = sq_pool.tile([P, 512], F32, tag=f"tmp{j}", name="xnt")
            nc.vector.tensor_mul(out=tmp, in0=x_tiles_by_nt[nt][j], in1=ps_a)
            nc.vector.tensor_add(out=xn_out_fn(nt, j), in0=tmp, in1=ps_c)


def _emit(tc, nc, x_fm, x_own_d, wqkv, bqk, wproj_d, bproj_d, wfc1_d, bfc1_d,
          wfc2_d, bfc2_d, out_fm):
    ctx_pools = []

    cons_pool = tc.alloc_tile_pool(name="cons", bufs=1)
    ctx_pools.append(cons_pool)
    ones2 = cons_pool.tile([P, 2], F32)
    nc.vector.memset(ones2, 1.0)
    ones2b = cons_pool.tile([P, 2], BF16)
    nc.vector.memset(ones2b, 1.0)
    half2 = cons_pool.tile([2, P], F32)
    nc.vector.memset(half2, 0.5)
    eps2 = cons_pool.tile([2, 1], F32)
    nc.vector.memset(eps2, EPS)

    bqk_sb = cons_pool.tile([P, 12], F32)
    nc.sync.dma_start(out=bqk_sb, in_=bqk)
    bproj_sb = cons_pool.tile([P, NJ], F32)
    nc.sync.dma_start(out=bproj_sb, in_=bproj_d)
    bfc1_sb = cons_pool.tile([P, NMLP], F32)
    nc.sync.dma_start(out=bfc1_sb, in_=bfc1_d)
    bfc2_sb = cons_pool.tile([P, NJ], F32)
    nc.sync.dma_start(out=bfc2_sb, in_=bfc2_d)

    stats = tc.alloc_tile_pool(name="stats", bufs=1)
    ctx_pools.append(stats)

    # k/q bf16; x_own fp32 residual; live until proj.
    persist = tc.alloc_tile_pool(name="persist", bufs=1)
    k_sb = [persist.tile([P, TB], BF16, tag=f"k{j}", name=f"k{j}")
            for j in range(NJ)]
    q_sb = [persist.tile([P, TO], BF16, tag=f"q{j}", name=f"q{j}")
            for j in range(NJ)]
    x_own = [persist.tile([P, TO], F32, tag=f"xo{j}", name=f"xo{j}")
             for j in range(NJ)]

    xn_pool = tc.alloc_tile_pool(name="xnpool", bufs=1)
    xn_all = [xn_pool.tile([P, TB], BF16, tag=f"xn{j}", name=f"xn{j}")
              for j in range(NJ)]

    # ---------------- Phase 1: load x (bf16), LN1 -> xn_all (bf16) ---------
    with (
        tc.tile_pool(name="xstream", bufs=1) as xpool,
        tc.tile_pool(name="sqpool", bufs=2) as sq_pool,
        tc.tile_pool(name="lnps", bufs=2, space="PSUM") as ln_ps,
        tc.tile_pool(name="bcps", bufs=2, space="PSUM") as bc_ps,
    ):
        for j in range(NJ):
            nc.sync.dma_start(out=x_own[j], in_=x_own_d[j * P:(j + 1) * P, :])
        x_by_nt = []
        for nt in range(NT):
            xt = [xpool.tile([P, TO], BF16, tag=f"xs{nt}_{j}",
                             name=f"xs{nt}_{j}") for j in range(NJ)]
            for j in range(NJ):
                nc.sync.dma_start(
                    out=xt[j],
                    in_=x_fm[j * P:(j + 1) * P, nt * TO:(nt + 1) * TO])
            x_by_nt.append(xt)
        pools = (tc, stats, sq_pool, ln_ps, bc_ps)
        _ln_batched(nc, pools, x_by_nt,
                    lambda nt, j: xn_all[j][:, nt * TO:(nt + 1) * TO],
                    ones2b, half2, eps2, NT, True)

    # ---------------- Phase 2: Q and K (bf16) ----------------
    with (
        tc.tile_pool(name="wkq", bufs=1) as wkq_pool,
        tc.tile_pool(name="mmps", bufs=4, space="PSUM") as mm_ps,
    ):
        wkq = []
        for j in range(NJ):
            t = wkq_pool.tile([P, 2 * D], BF16, tag=f"wkq{j}", name=f"wkq{j}")
            nc.sync.dma_start(out=t, in_=wqkv[j * P:(j + 1) * P, 0:2 * D])
            wkq.append(t)
        for m in range(NJ):
            pt = mm_ps.tile([P, TO], F32, tag="mm", name="mmq")
            for j in range(NJ):
                nc.tensor.matmul(pt[:], lhsT=wkq[j][:, m * P:(m + 1) * P],
                                 rhs=xn_all[j][:, 0:TO],
                                 start=(j == 0), stop=(j == NJ - 1))
            nc.vector.tensor_scalar_add(q_sb[m], pt, bqk_sb[:, m:m + 1])
        for m in range(NJ):
            for nt in range(NT):
                pt = mm_ps.tile([P, TO], F32, tag="mm", name="mmk")
                for j in range(NJ):
                    nc.tensor.matmul(
                        pt[:], lhsT=wkq[j][:, D + m * P:D + (m + 1) * P],
                        rhs=xn_all[j][:, nt * TO:(nt + 1) * TO],
                        start=(j == 0), stop=(j == NJ - 1))
                nc.vector.tensor_scalar_add(
                    k_sb[m][:, nt * TO:(nt + 1) * TO], pt,
                    bqk_sb[:, 6 + m:7 + m])

    # ------------- Phase 3: V token-major bf16, with ones columns ----------
    v_pool = tc.alloc_tile_pool(name="vpool", bufs=1, side="right")
    v_sb = [v_pool.tile([P, NH * VW], BF16, tag=f"v{t}", name=f"v{t}")
            for t in range(NTK)]
    with (
        tc.tile_pool(name="wv", bufs=1) as wv_pool,
        tc.tile_pool(name="vps5", bufs=2, space="PSUM") as v_ps5,
        tc.tile_pool(name="vps2", bufs=2, space="PSUM") as v_ps2,
    ):
        wv = []
        for j in range(NJ):
            t = wv_pool.tile([P, D], BF16, tag=f"wv{j}", name=f"wv{j}")
            nc.sync.dma_start(out=t, in_=wqkv[j * P:(j + 1) * P, 2 * D:3 * D])
            wv.append(t)
        for mt in range(NTK):
            vt = v_sb[mt]
            nc.vector.memset(
                vt.rearrange("p (h w) -> p h w", w=VW)[:, :, 64:66], 1.0)
            pt5 = v_ps5.tile([P, 512], F32, tag="v5", name="v5")
            pt2 = v_ps2.tile([P, 256], F32, tag="v2", name="v2")
            for j in range(NJ):
                lhs = xn_all[j][:, mt * P:(mt + 1) * P]
                nc.tensor.matmul(pt5[:], lhsT=lhs, rhs=wv[j][:, 0:512],
                                 start=(j == 0), stop=(j == NJ - 1))
            for j in range(NJ):
                lhs = xn_all[j][:, mt * P:(mt + 1) * P]
                nc.tensor.matmul(pt2[:], lhsT=lhs, rhs=wv[j][:, 512:768],
                                 start=(j == 0), stop=(j == NJ - 1))
            v3 = vt.rearrange("p (h w) -> p h w", w=VW)
            nc.vector.tensor_copy(
                out=v3[:, 0:8, 0:64],
                in_=pt5.rearrange("p (h w) -> p h w", w=64))
            nc.vector.tensor_copy(
                out=v3[:, 8:12, 0:64],
                in_=pt2.rearrange("p (h w) -> p h w", w=64))
    xn_pool.release()

    # ---------------- Phase 4: attention ----------------
    attn_pool = tc.alloc_tile_pool(name="attnpool", bufs=1)
    attn_fm = [attn_pool.tile([P, TO], BF16, tag=f"at{j}", name=f"at{j}")
               for j in range(NJ)]
    wp_pool = tc.alloc_tile_pool(name="wproj", bufs=1)
    wp = []
    for j in range(NJ):
        t = wp_pool.tile([P, D], BF16, tag=f"wp{j}", name=f"wp{j}")
        nc.sync.dma_start(out=t, in_=wproj_d[j * P:(j + 1) * P, :])
        wp.append(t)
    with (
        tc.tile_pool(name="seps", bufs=3, space="PSUM") as se_ps,
        tc.tile_pool(name="avps", bufs=1, space="PSUM") as av_ps,
        tc.tile_pool(name="sesb", bufs=4) as se_pool,
        tc.tile_pool(name="bcsb", bufs=2) as bc_pool,
    ):
        for hp in range(NJ):
            pt_av_a = av_ps.tile([P, 512], F32, tag="ava", name="ava")
            pt_av_b = av_ps.tile([P, 512], F32, tag="avb", name="avb")
            for tk2 in range(NTK // 2):
                ps_a = se_ps.tile([P, 1024], F32, tag="se", name="psea")
                ps_b = se_ps.tile([P, 1024], F32, tag="se", name="pseb")
                for half in range(2):
                    tk = 2 * tk2 + half
                    ksl = slice(tk * P, (tk + 1) * P)
                    fsl = slice(half * 512, (half + 1) * 512)
                    nc.tensor.matmul(ps_a[:, fsl],
                                     lhsT=k_sb[hp][0:64, ksl],
                                     rhs=q_sb[hp][0:64, :],
                                     start=True, stop=True)
                    nc.tensor.matmul(ps_b[:, fsl],
                                     lhsT=k_sb[hp][64:128, ksl],
                                     rhs=q_sb[hp][64:128, :],
                                     start=True, stop=True)
                se_a = se_pool.tile([P, 1024], BF16, tag="sea", name="sea")
                se_b = se_pool.tile([P, 1024], BF16, tag="seb", name="seb")
                nc.scalar.activation(out=se_a, in_=ps_a, func=AF.Exp)
                nc.scalar.activation(out=se_b, in_=ps_b, func=AF.Exp)
                for half in range(2):
                    tk = 2 * tk2 + half
                    fsl = slice(half * 512, (half + 1) * 512)
                    first = (tk == 0)
                    last = (tk == NTK - 1)
                    nc.tensor.matmul(
                        pt_av_a[:VW, :],
                        lhsT=v_sb[tk][:, (2 * hp) * VW:(2 * hp + 1) * VW],
                        rhs=se_a[:, fsl], start=first, stop=last)
                    nc.tensor.matmul(
                        pt_av_b[:VW, :],
                        lhsT=v_sb[tk][:, (2 * hp + 1) * VW:(2 * hp + 2) * VW],
                        rhs=se_b[:, fsl], start=first, stop=last)
            for head, pt_av in ((0, pt_av_a), (1, pt_av_b)):
                # DVE is idle during the (ACT-bound) exp stream; reciprocal
                # here keeps ACT's FIFO free of waits on av completion.
                rec = bc_pool.tile([2, 512], F32, tag="rec", name="rec")
                nc.vector.reciprocal(out=R(rec), in_=pt_av[64:66, :])
                ps_bc = se_ps.tile([64, 512], F32, tag="se", name="psbc")
                nc.tensor.matmul(ps_bc[:], lhsT=R(half2[:, 0:64]), rhs=R(rec),
                                 start=True, stop=True)
                bc_sb = bc_pool.tile([64, 512], F32, tag="bc", name="bcsb")
                nc.vector.tensor_copy(out=bc_sb, in_=ps_bc)
                nc.vector.tensor_mul(
                    out=attn_fm[hp][head * 64:(head + 1) * 64, :],
                    in0=pt_av[0:64, :], in1=bc_sb)
    v_pool.release()

    # ---------------- Phase 5: proj + residual -> x2 (fp32) ----------------
    x2_pool = tc.alloc_tile_pool(name="x2pool", bufs=1, side="right")
    ctx_pools.append(x2_pool)
    x2_sb = [x2_pool.tile([P, TO], F32, tag=f"x2{j}", name=f"x2{j}")
             for j in range(NJ)]
    wfc1_pool = tc.alloc_tile_pool(name="wfc1", bufs=1, side="right")
    ctx_pools.append(wfc1_pool)
    wf1 = []
    for j in range(NJ):
        t = wfc1_pool.tile([P, DFF], BF16, tag=f"wf1{j}", name=f"wf1{j}")
        nc.sync.dma_start(out=t, in_=wfc1_d[j * P:(j + 1) * P, :])
        wf1.append(t)
    with (
        tc.tile_pool(name="mmps2", bufs=3, space="PSUM") as mm_ps2,
    ):
        for m in range(NJ):
            pt = mm_ps2.tile([P, TO], F32, tag="mm", name="mmproj")
            for j in range(NJ):
                nc.tensor.matmul(pt[:], lhsT=wp[j][:, m * P:(m + 1) * P],
                                 rhs=attn_fm[j],
                                 start=(j == 0), stop=(j == NJ - 1))
            nc.vector.scalar_tensor_tensor(
                out=R(x2_sb[m]), in0=pt, scalar=bproj_sb[:, m:m + 1],
                in1=x_own[m], op0=ALU.add, op1=ALU.add)
    wp_pool.release()
    attn_pool.release()
    persist.release()

    # ---------------- Phase 6: LN2 -> h (bf16) ----------------
    h_pool = tc.alloc_tile_pool(name="hpool", bufs=1, side="right")
    ctx_pools.append(h_pool)
    h_sb = [h_pool.tile([P, TO], BF16, tag=f"h{j}", name=f"h{j}")
            for j in range(NJ)]
    with (
        tc.tile_pool(name="sqpool2", bufs=2) as sq_pool2,
        tc.tile_pool(name="lnps2", bufs=1, space="PSUM") as ln_ps2,
        tc.tile_pool(name="bcps3", bufs=1, space="PSUM") as bc_ps3,
    ):
        pools = (tc, stats, sq_pool2, ln_ps2, bc_ps3)
        _ln_batched(nc, pools, [x2_sb],
                    lambda nt, j: h_sb[j][:, :],
                    ones2, half2, eps2, 1, False)

    # ---------------- Phase 7: fc1 + gelu -> h1 (bf16) ----------------
    h1_pool = tc.alloc_tile_pool(name="h1", bufs=1, side="right")
    ctx_pools.append(h1_pool)
    h1_sb = [h1_pool.tile([P, TO], BF16, tag=f"h1{m}", name=f"h1{m}")
             for m in range(NMLP)]
    with (
        tc.tile_pool(name="mmps3", bufs=4, space="PSUM") as mm_ps3,
    ):
        for m in range(NMLP):
            pt = mm_ps3.tile([P, TO], F32, tag="mm", name="mmfc1")
            for j in range(NJ):
                nc.tensor.matmul(pt[:], lhsT=wf1[j][:, m * P:(m + 1) * P],
                                 rhs=h_sb[j],
                                 start=(j == 0), stop=(j == NJ - 1))
            nc.scalar.activation(out=h1_sb[m], in_=pt, func=AF.Gelu,
                                 bias=bfc1_sb[:, m:m + 1])

    # ---------------- Phase 8: fc2 + residual + store ----------------
    with (
        tc.tile_pool(name="wfc2", bufs=6) as wfc2_pool,
        tc.tile_pool(name="fc2ps", bufs=1, space="PSUM") as fc2_ps,
        tc.tile_pool(name="outsb", bufs=2) as out_pool,
    ):
        pts = [fc2_ps.tile([P, TO], F32, tag=f"fc2_{m}", name=f"fc2_{m}")
               for m in range(NJ)]
        for j in range(NMLP):
            wt = wfc2_pool.tile([P, D], BF16, tag="wf2", name="wf2")
            nc.sync.dma_start(out=wt, in_=wfc2_d[j * P:(j + 1) * P, :])
            for m in range(NJ):
                nc.tensor.matmul(pts[m][:], lhsT=wt[:, m * P:(m + 1) * P],
                                 rhs=h1_sb[j],
                                 start=(j == 0), stop=(j == NMLP - 1))
        for m in range(NJ):
            ot = out_pool.tile([P, TO], F32, tag="out", name="ot")
            nc.vector.scalar_tensor_tensor(
                out=ot, in0=pts[m], scalar=bfc2_sb[:, m:m + 1],
                in1=x2_sb[m], op0=ALU.add, op1=ALU.add)
            nc.sync.dma_start(out=out_fm[m * P:(m + 1) * P, :], in_=ot)

    for pool in reversed(ctx_pools):
        pool.release()


_NC_CACHE = {}


def _get_nc():
    if "nc" not in _NC_CACHE:
        _NC_CACHE["nc"] = _build()
    return _NC_CACHE["nc"]


def _host_prep(inputs):
    f32 = lambda a: np.ascontiguousarray(np.asarray(a, dtype=np.float32))
    x = f32(inputs["x"])            # [2, 2048, 768]
    W_qkv = f32(inputs["W_qkv"])    # [768, 2304]
    b_qkv = f32(inputs["b_qkv"])
    W_proj = f32(inputs["W_proj"])
    b_proj = f32(inputs["b_proj"])
    W_fc1 = f32(inputs["W_fc1"])
    b_fc1 = f32(inputs["b_fc1"])
    W_fc2 = f32(inputs["W_fc2"])
    b_fc2 = f32(inputs["b_fc2"])
    ln1_g = f32(inputs["ln1_g"])
    ln1_b = f32(inputs["ln1_b"])
    ln2_g = f32(inputs["ln2_g"])
    ln2_b = f32(inputs["ln2_b"])

    scale = DH ** -0.5
    wqkv_eff = W_qkv * ln1_g[:, None]
    bqkv_eff = ln1_b @ W_qkv + b_qkv
    wqkv_eff[:, :D] *= scale
    bqkv_eff_q = bqkv_eff[:D] * scale
    bqk = np.concatenate([bqkv_eff_q, bqkv_eff[D:2 * D]]).astype(np.float32)
    bv = bqkv_eff[2 * D:]
    bproj_eff = (b_proj + bv @ W_proj).astype(np.float32)
    wfc1_eff = (W_fc1 * ln2_g[:, None]).astype(np.float32)
    bfc1_eff = (ln2_b @ W_fc1 + b_fc1).astype(np.float32)

    bf = lambda a: np.ascontiguousarray(a.astype(ml_dtypes.bfloat16))
    pack = lambda b: np.ascontiguousarray(
        b.reshape(-1, P).T.astype(np.float32))
    shared = {
        "wqkv": bf(wqkv_eff),
        "bqk": pack(bqk),
        "wproj": bf(W_proj),
        "bproj": pack(bproj_eff),
        "wfc1": bf(wfc1_eff),
        "bfc1": pack(bfc1_eff),
        "wfc2": bf(W_fc2),
        "bfc2": pack(b_fc2),
    }
    in_maps = []
    for c in range(N_CORES):
        b, q = divmod(c, 4)
        xb = np.roll(x[b], -TO * q, axis=0)  # own tokens at rows 0:TO
        m = dict(shared)
        m["x_fm"] = bf(xb.T)
        m["x_own"] = np.ascontiguousarray(xb[:TO].T)
        in_maps.append(m)
    return in_maps


def _run(inputs, trace=False):
    nc = _get_nc()
    in_maps = _host_prep(inputs)
    res = bass_utils.run_bass_kernel_spmd(nc, in_maps, list(range(N_CORES)),
                                          trace=trace)
    B = 2
    out = np.empty((B, TB, D), dtype=np.float32)
    for c in range(N_CORES):
        b, q = divmod(c, 4)
        out[b, TO * q:TO * (q + 1), :] = res.results[c]["out_fm"].T
    return out, res


def kernel(**inputs):
    out, _ = _run(inputs, trace=False)
    return out


if __name__ == "__main__":
    print("building...")
    _get_nc()
    print("built ok")


# revision 16
# speedup vs baseline: 1.0130x; 1.0130x over previous
"""Trainium2 Bass kernel for a dense transformer block (pre-LN, MHA + GELU MLP).

Sharding: 8 cores = 2 batches x 4 sequence-quarters. Each core recomputes
LN1 + K/V for its full batch (zero cross-core communication), and computes
Q/attention/proj/MLP for its own 512 tokens only.

Device works feature-major ([feature, token]); the host pre-transposes x and
post-transposes the output. LN gains/biases are folded into the following
matmul weights on the host; the qk scale (1/8) is folded into W_q; the v bias
is folded into b_proj.

Numerics: matmul operands are bf16 (fp32 PSUM accumulation); the residual
stream (x, x2, out), layernorm statistics, and softmax denominators stay fp32.
LN-statistic / broadcast matmuls run in fp32r.
"""
import sys

sys.path.insert(0, "/opt/trn_rl_repo")

import numpy as np
import ml_dtypes

import concourse.bass as bass  # noqa: F401
import concourse.tile as tile
from concourse import bacc, mybir, bass_utils

F32 = mybir.dt.float32
F32R = mybir.dt.float32r
BF16 = mybir.dt.bfloat16
AF = mybir.ActivationFunctionType
ALU = mybir.AluOpType

P = 128
D = 768
NH = 12
DH = 64
DFF = 3072
TB = 2048      # tokens per batch
TO = 512       # tokens owned per core
NJ = D // P    # 6 feature tiles
NT = TB // TO  # 4 token tiles per batch
NTK = TB // P  # 16 key tiles
NMLP = DFF // P  # 24
EPS = 1e-6
N_CORES = 8
VW = 66        # 64 v cols + 2 ones cols per head


def R(ap):
    return ap.bitcast(F32R)


def _build():
    nc = bacc.Bacc("TRN2", target_bir_lowering=False, debug=False,
                   num_devices=N_CORES)

    x_fm = nc.dram_tensor("x_fm", [D, TB], BF16, kind="ExternalInput").ap()
    x_own_d = nc.dram_tensor("x_own", [D, TO], F32, kind="ExternalInput").ap()
    wqkv = nc.dram_tensor("wqkv", [D, 3 * D], BF16, kind="ExternalInput").ap()
    bqk = nc.dram_tensor("bqk", [P, 12], F32, kind="ExternalInput").ap()
    wproj = nc.dram_tensor("wproj", [D, D], BF16, kind="ExternalInput").ap()
    bproj = nc.dram_tensor("bproj", [P, NJ], F32, kind="ExternalInput").ap()
    wfc1 = nc.dram_tensor("wfc1", [D, DFF], BF16, kind="ExternalInput").ap()
    bfc1 = nc.dram_tensor("bfc1", [P, NMLP], F32, kind="ExternalInput").ap()
    wfc2 = nc.dram_tensor("wfc2", [DFF, D], BF16, kind="ExternalInput").ap()
    bfc2 = nc.dram_tensor("bfc2", [P, NJ], F32, kind="ExternalInput").ap()
    out_fm = nc.dram_tensor("out_fm", [D, TO], F32, kind="ExternalOutput").ap()

    with nc.allow_low_precision(reason="bf16 matmul operands are intentional"), \
            tile.TileContext(nc) as tc:
        _emit(tc, nc, x_fm, x_own_d, wqkv, bqk, wproj, bproj, wfc1, bfc1,
              wfc2, bfc2, out_fm)
    nc.compile()
    return nc


def _ln_batched(nc, pools, x_tiles_by_nt, xn_out_fn, ones_t, half2, eps2,
                n_nt, bf16_in):
    """LayerNorm over n_nt token tiles of 512, one batched stats chain.

    x_tiles_by_nt[nt][j]: input tiles [128, 512] (bf16 if bf16_in else fp32).
    xn_out_fn(nt, j) -> bf16 dest AP [128, 512].
    """
    tc, stats, sq_pool, ln_ps, bc_ps = pools
    ntot = 512 * n_nt
    sum_sb = stats.tile([2, ntot], F32, tag="sum_sb", name="sum_sb")
    sq_sb = stats.tile([2, ntot], F32, tag="sq_sb", name="sq_sb")
    mk = (lambda ap: ap) if bf16_in else R
    sqdt = BF16 if bf16_in else F32
    for nt in range(n_nt):
        x_tiles = x_tiles_by_nt[nt]
        xsq = []
        for j in range(NJ):
            t = sq_pool.tile([P, 512], sqdt, tag=f"xsq{j}", name="xsqt")
            nc.scalar.activation(out=mk(t), in_=x_tiles[j], func=AF.Square)
            xsq.append(t)
        ps_sum = ln_ps.tile([2, 512], F32, tag="lnsum", name="ps_sum")
        ps_sq = ln_ps.tile([2, 512], F32, tag="lnsq", name="ps_sq")
        for j in range(NJ):
            nc.tensor.matmul(ps_sum[:], lhsT=mk(ones_t), rhs=mk(x_tiles[j]),
                             start=(j == 0), stop=(j == NJ - 1))
        for j in range(NJ):
            nc.tensor.matmul(ps_sq[:], lhsT=mk(ones_t), rhs=mk(xsq[j]),
                             start=(j == 0), stop=(j == NJ - 1))
        sl = slice(nt * 512, (nt + 1) * 512)
        nc.vector.tensor_copy(out=sum_sb[:, sl], in_=ps_sum)
        nc.vector.tensor_copy(out=sq_sb[:, sl], in_=ps_sq)
    # var*D^2 = D*sumsq - sum^2 ; rs = exp(-0.5*ln(varD2/D^2 + eps))
    t1 = stats.tile([2, ntot], F32, tag="t1", name="t1")
    nc.vector.scalar_tensor_tensor(out=t1, in0=sum_sb, scalar=-1.0,
                                   in1=sum_sb, op0=ALU.mult, op1=ALU.mult)
    nc.vector.scalar_tensor_tensor(out=t1, in0=sq_sb, scalar=float(D),
                                   in1=t1, op0=ALU.mult, op1=ALU.add)
    nc.scalar.activation(out=t1, in_=t1, func=AF.Ln, bias=eps2,
                         scale=1.0 / (D * D))
    rs = stats.tile([2, ntot], F32, tag="rs", name="rs")
    nc.scalar.activation(out=R(rs), in_=t1, func=AF.Exp, scale=-0.5)
    cc = stats.tile([2, ntot], F32, tag="cc", name="cc")
    nc.vector.scalar_tensor_tensor(out=R(cc), in0=sum_sb, scalar=-1.0 / D,
                                   in1=rs, op0=ALU.mult, op1=ALU.mult)
    for nt in range(n_nt):
        sl = slice(nt * 512, (nt + 1) * 512)
        ps_a = bc_ps.tile([P, 512], F32, tag="bca", name="ps_a")
        nc.tensor.matmul(ps_a[:], lhsT=R(half2), rhs=R(rs[:, sl]),
                         start=True, stop=True)
        ps_c = bc_ps.tile([P, 512], F32, tag="bcc", name="ps_c")
        nc.tensor.matmul(ps_c[:], lhsT=R(half2), rhs=R(cc[:, sl]),
                         start=True, stop=True)
        for j in range(NJ):
            tmp = sq_pool.tile([P, 512], F32, tag=f"tmp{j}", name="xnt")
            nc.vector.tensor_mul(out=tmp, in0=x_tiles_by_nt[nt][j], in1=ps_a)
            nc.vector.tensor_add(out=xn_out_fn(nt, j), in0=tmp, in1=ps_c)


def _emit(tc, nc, x_fm, x_own_d, wqkv, bqk, wproj_d, bproj_d, wfc1_d, bfc1_d,
          wfc2_d, bfc2_d, out_fm):
    ctx_pools = []

    cons_pool = tc.alloc_tile_pool(name="cons", bufs=1)
    ctx_pools.append(cons_pool)
    ones2 = cons_pool.tile([P, 2], F32)
    nc.vector.memset(ones2, 1.0)
    ones2b = cons_pool.tile([P, 2], BF16)
    nc.vector.memset(ones2b, 1.0)
    half2 = cons_pool.tile([2, P], F32)
    nc.vector.memset(half2, 0.5)
    eps2 = cons_pool.tile([2, 1], F32)
    nc.vector.memset(eps2, EPS)

    bqk_sb = cons_pool.tile([P, 12], F32)
    nc.sync.dma_start(out=bqk_sb, in_=bqk)
    bproj_sb = cons_pool.tile([P, NJ], F32)
    nc.sync.dma_start(out=bproj_sb, in_=bproj_d)
    bfc1_sb = cons_pool.tile([P, NMLP], F32)
    nc.sync.dma_start(out=bfc1_sb, in_=bfc1_d)
    bfc2_sb = cons_pool.tile([P, NJ], F32)
    nc.sync.dma_start(out=bfc2_sb, in_=bfc2_d)

    stats = tc.alloc_tile_pool(name="stats", bufs=1)
    ctx_pools.append(stats)

    # k/q bf16; x_own fp32 residual; live until proj.
    persist = tc.alloc_tile_pool(name="persist", bufs=1)
    k_sb = [persist.tile([P, TB], BF16, tag=f"k{j}", name=f"k{j}")
            for j in range(NJ)]
    q_sb = [persist.tile([P, TO], BF16, tag=f"q{j}", name=f"q{j}")
            for j in range(NJ)]
    x_own = [persist.tile([P, TO], F32, tag=f"xo{j}", name=f"xo{j}")
             for j in range(NJ)]

    v_pool = tc.alloc_tile_pool(name="vpool", bufs=1, side="right")
    v_sb = [v_pool.tile([P, NH * VW], BF16, tag=f"v{t}", name=f"v{t}")
            for t in range(NTK)]

    xn_pool = tc.alloc_tile_pool(name="xnpool", bufs=1)
    xn_all = [xn_pool.tile([P, TB], BF16, tag=f"xn{j}", name=f"xn{j}")
              for j in range(NJ)]

    # ---------------- Phase 1: load x (bf16), LN1 -> xn_all (bf16) ---------
    with (
        tc.tile_pool(name="xstream", bufs=1) as xpool,
        tc.tile_pool(name="sqpool", bufs=2) as sq_pool,
        tc.tile_pool(name="lnps", bufs=2, space="PSUM") as ln_ps,
        tc.tile_pool(name="bcps", bufs=2, space="PSUM") as bc_ps,
    ):
        for j in range(NJ):
            nc.sync.dma_start(out=x_own[j], in_=x_own_d[j * P:(j + 1) * P, :])
        x_by_nt = []
        for nt in range(NT):
            xt = [xpool.tile([P, TO], BF16, tag=f"xs{nt}_{j}",
                             name=f"xs{nt}_{j}") for j in range(NJ)]
            for j in range(NJ):
                nc.sync.dma_start(
                    out=xt[j],
                    in_=x_fm[j * P:(j + 1) * P, nt * TO:(nt + 1) * TO])
            x_by_nt.append(xt)
        pools = (tc, stats, sq_pool, ln_ps, bc_ps)
        _ln_batched(nc, pools, x_by_nt,
                    lambda nt, j: xn_all[j][:, nt * TO:(nt + 1) * TO],
                    ones2b, half2, eps2, NT, True)

    # ---------------- Phase 2: Q, V, then K (bf16) ----------------
    # V is emitted before K so that attention (gated on K) starts only after
    # V is resident; av accumulation then never convoys behind the V matmuls.
    with (
        tc.tile_pool(name="wkq", bufs=1) as wkq_pool,
        tc.tile_pool(name="mmps", bufs=4, space="PSUM") as mm_ps,
        tc.tile_pool(name="wv", bufs=1) as wv_pool,
        tc.tile_pool(name="vps5", bufs=2, space="PSUM") as v_ps5,
    ):
        wkq = []
        for j in range(NJ):
            t = wkq_pool.tile([P, 2 * D], BF16, tag=f"wkq{j}", name=f"wkq{j}")
            nc.sync.dma_start(out=t, in_=wqkv[j * P:(j + 1) * P, 0:2 * D])
            wkq.append(t)
        wv = []
        for j in range(NJ):
            t = wv_pool.tile([P, D], BF16, tag=f"wv{j}", name=f"wv{j}")
            nc.sync.dma_start(out=t, in_=wqkv[j * P:(j + 1) * P, 2 * D:3 * D])
            wv.append(t)
        # Q for own tokens
        for m in range(NJ):
            pt = mm_ps.tile([P, TO], F32, tag="mm", name="mmq")
            for j in range(NJ):
                nc.tensor.matmul(pt[:], lhsT=wkq[j][:, m * P:(m + 1) * P],
                                 rhs=xn_all[j][:, 0:TO],
                                 start=(j == 0), stop=(j == NJ - 1))
            nc.vector.tensor_scalar_add(q_sb[m], pt, bqk_sb[:, m:m + 1])
        # V (token-major with ones columns)
        for mt in range(NTK):
            vt = v_sb[mt]
            nc.vector.memset(
                vt.rearrange("p (h w) -> p h w", w=VW)[:, :, 64:66], 1.0)
            pt5 = v_ps5.tile([P, 512], F32, tag="v5", name="v5")
            pt2 = mm_ps.tile([P, TO], F32, tag="mm", name="v2")
            for j in range(NJ):
                lhs = xn_all[j][:, mt * P:(mt + 1) * P]
                nc.tensor.matmul(pt5[:], lhsT=lhs, rhs=wv[j][:, 0:512],
                                 start=(j == 0), stop=(j == NJ - 1))
            for j in range(NJ):
                lhs = xn_all[j][:, mt * P:(mt + 1) * P]
                nc.tensor.matmul(pt2[:, 0:256], lhsT=lhs, rhs=wv[j][:, 512:768],
                                 start=(j == 0), stop=(j == NJ - 1))
            v3 = vt.rearrange("p (h w) -> p h w", w=VW)
            nc.vector.tensor_copy(
                out=v3[:, 0:8, 0:64],
                in_=pt5.rearrange("p (h w) -> p h w", w=64))
            nc.vector.tensor_copy(
                out=v3[:, 8:12, 0:64],
                in_=pt2[:, 0:256].rearrange("p (h w) -> p h w", w=64))
        # K for all tokens
        for m in range(NJ):
            for nt in range(NT):
                pt = mm_ps.tile([P, TO], F32, tag="mm", name="mmk")
                for j in range(NJ):
                    nc.tensor.matmul(
                        pt[:], lhsT=wkq[j][:, D + m * P:D + (m + 1) * P],
                        rhs=xn_all[j][:, nt * TO:(nt + 1) * TO],
                        start=(j == 0), stop=(j == NJ - 1))
                nc.vector.tensor_scalar_add(
                    k_sb[m][:, nt * TO:(nt + 1) * TO], pt,
                    bqk_sb[:, 6 + m:7 + m])
    xn_pool.release()

    # ---------------- Phase 4: attention ----------------
    attn_pool = tc.alloc_tile_pool(name="attnpool", bufs=1)
    attn_fm = [attn_pool.tile([P, TO], BF16, tag=f"at{j}", name=f"at{j}")
               for j in range(NJ)]
    wp_pool = tc.alloc_tile_pool(name="wproj", bufs=1)
    wp = []
    for j in range(NJ):
        t = wp_pool.tile([P, D], BF16, tag=f"wp{j}", name=f"wp{j}")
        nc.sync.dma_start(out=t, in_=wproj_d[j * P:(j + 1) * P, :])
        wp.append(t)
    with (
        tc.tile_pool(name="seps", bufs=3, space="PSUM") as se_ps,
        tc.tile_pool(name="avps", bufs=1, space="PSUM") as av_ps,
        tc.tile_pool(name="sesb", bufs=6) as se_pool,
        tc.tile_pool(name="bcsb", bufs=2) as bc_pool,
    ):
        for hp in range(NJ):
            pt_av_a = av_ps.tile([P, 512], F32, tag="ava", name="ava")
            pt_av_b = av_ps.tile([P, 512], F32, tag="avb", name="avb")
            for tk2 in range(NTK // 2):
                ps_a = se_ps.tile([P, 1024], F32, tag="se", name="psea")
                ps_b = se_ps.tile([P, 1024], F32, tag="se", name="pseb")
                for half in range(2):
                    tk = 2 * tk2 + half
                    ksl = slice(tk * P, (tk + 1) * P)
                    fsl = slice(half * 512, (half + 1) * 512)
                    nc.tensor.matmul(ps_a[:, fsl],
                                     lhsT=k_sb[hp][0:64, ksl],
                                     rhs=q_sb[hp][0:64, :],
                                     start=True, stop=True)
                    nc.tensor.matmul(ps_b[:, fsl],
                                     lhsT=k_sb[hp][64:128, ksl],
                                     rhs=q_sb[hp][64:128, :],
                                     start=True, stop=True)
                se_a = se_pool.tile([P, 1024], BF16, tag="sea", name="sea")
                se_b = se_pool.tile([P, 1024], BF16, tag="seb", name="seb")
                nc.scalar.activation(out=se_a, in_=ps_a, func=AF.Exp)
                nc.scalar.activation(out=se_b, in_=ps_b, func=AF.Exp)
                for half in range(2):
                    tk = 2 * tk2 + half
                    fsl = slice(half * 512, (half + 1) * 512)
                    first = (tk == 0)
                    last = (tk == NTK - 1)
                    nc.tensor.matmul(
                        pt_av_a[:VW, :],
                        lhsT=v_sb[tk][:, (2 * hp) * VW:(2 * hp + 1) * VW],
                        rhs=se_a[:, fsl], start=first, stop=last)
                    nc.tensor.matmul(
                        pt_av_b[:VW, :],
                        lhsT=v_sb[tk][:, (2 * hp + 1) * VW:(2 * hp + 2) * VW],
                        rhs=se_b[:, fsl], start=first, stop=last)
            for head, pt_av in ((0, pt_av_a), (1, pt_av_b)):
                # DVE is idle during the (ACT-bound) exp stream; reciprocal
                # here keeps ACT's FIFO free of waits on av completion.
                rec = bc_pool.tile([2, 512], F32, tag="rec", name="rec")
                nc.vector.reciprocal(out=R(rec), in_=pt_av[64:66, :])
                ps_bc = se_ps.tile([64, 512], F32, tag="se", name="psbc")
                nc.tensor.matmul(ps_bc[:], lhsT=R(half2[:, 0:64]), rhs=R(rec),
                                 start=True, stop=True)
                bc_sb = bc_pool.tile([64, 512], F32, tag="bc", name="bcsb")
                nc.vector.tensor_copy(out=bc_sb, in_=ps_bc)
                nc.vector.tensor_mul(
                    out=attn_fm[hp][head * 64:(head + 1) * 64, :],
                    in0=pt_av[0:64, :], in1=bc_sb)
    v_pool.release()

    # ---------------- Phase 5: proj + residual -> x2 (fp32) ----------------
    x2_pool = tc.alloc_tile_pool(name="x2pool", bufs=1, side="right")
    ctx_pools.append(x2_pool)
    x2_sb = [x2_pool.tile([P, TO], F32, tag=f"x2{j}", name=f"x2{j}")
             for j in range(NJ)]
    wfc1_pool = tc.alloc_tile_pool(name="wfc1", bufs=1, side="right")
    ctx_pools.append(wfc1_pool)
    wf1 = []
    for j in range(NJ):
        t = wfc1_pool.tile([P, DFF], BF16, tag=f"wf1{j}", name=f"wf1{j}")
        nc.sync.dma_start(out=t, in_=wfc1_d[j * P:(j + 1) * P, :])
        wf1.append(t)
    with (
        tc.tile_pool(name="mmps2", bufs=3, space="PSUM") as mm_ps2,
    ):
        for m in range(NJ):
            pt = mm_ps2.tile([P, TO], F32, tag="mm", name="mmproj")
            for j in range(NJ):
                nc.tensor.matmul(pt[:], lhsT=wp[j][:, m * P:(m + 1) * P],
                                 rhs=attn_fm[j],
                                 start=(j == 0), stop=(j == NJ - 1))
            nc.vector.scalar_tensor_tensor(
                out=R(x2_sb[m]), in0=pt, scalar=bproj_sb[:, m:m + 1],
                in1=x_own[m], op0=ALU.add, op1=ALU.add)
    wp_pool.release()
    attn_pool.release()
    persist.release()

    # ---------------- Phase 6: LN2 -> h (bf16) ----------------
    h_pool = tc.alloc_tile_pool(name="hpool", bufs=1, side="right")
    ctx_pools.append(h_pool)
    h_sb = [h_pool.tile([P, TO], BF16, tag=f"h{j}", name=f"h{j}")
            for j in range(NJ)]
    with (
        tc.tile_pool(name="sqpool2", bufs=2) as sq_pool2,
        tc.tile_pool(name="lnps2", bufs=1, space="PSUM") as ln_ps2,
        tc.tile_pool(name="bcps3", bufs=1, space="PSUM") as bc_ps3,
    ):
        pools = (tc, stats, sq_pool2, ln_ps2, bc_ps3)
        _ln_batched(nc, pools, [x2_sb],
                    lambda nt, j: h_sb[j][:, :],
                    ones2, half2, eps2, 1, False)

    # ---------------- Phase 7: fc1 + gelu -> h1 (bf16) ----------------
    h1_pool = tc.alloc_tile_pool(name="h1", bufs=1, side="right")
    ctx_pools.append(h1_pool)
    h1_sb = [h1_pool.tile([P, TO], BF16, tag=f"h1{m}", name=f"h1{m}")
             for m in range(NMLP)]
    with (
        tc.tile_pool(name="mmps3", bufs=4, space="PSUM") as mm_ps3,
    ):
        for m in range(NMLP):
            pt = mm_ps3.tile([P, TO], F32, tag="mm", name="mmfc1")
            for j in range(NJ):
                nc.tensor.matmul(pt[:], lhsT=wf1[j][:, m * P:(m + 1) * P],
                                 rhs=h_sb[j],
                                 start=(j == 0), stop=(j == NJ - 1))
            nc.scalar.activation(out=h1_sb[m], in_=pt, func=AF.Gelu,
                                 bias=bfc1_sb[:, m:m + 1])

    # ---------------- Phase 8: fc2 + residual + store ----------------
    with (
        tc.tile_pool(name="wfc2", bufs=6) as wfc2_pool,
        tc.tile_pool(name="fc2ps", bufs=1, space="PSUM") as fc2_ps,
        tc.tile_pool(name="outsb", bufs=2) as out_pool,
    ):
        pts = [fc2_ps.tile([P, TO], F32, tag=f"fc2_{m}", name=f"fc2_{m}")
               for m in range(NJ)]
        for j in range(NMLP):
            wt = wfc2_pool.tile([P, D], BF16, tag="wf2", name="wf2")
            nc.sync.dma_start(out=wt, in_=wfc2_d[j * P:(j + 1) * P, :])
            for m in range(NJ):
                nc.tensor.matmul(pts[m][:], lhsT=wt[:, m * P:(m + 1) * P],
                                 rhs=h1_sb[j],
                                 start=(j == 0), stop=(j == NMLP - 1))
        for m in range(NJ):
            ot = out_pool.tile([P, TO], F32, tag="out", name="ot")
            nc.vector.scalar_tensor_tensor(
                out=ot, in0=pts[m], scalar=bfc2_sb[:, m:m + 1],
                in1=x2_sb[m], op0=ALU.add, op1=ALU.add)
            nc.sync.dma_start(out=out_fm[m * P:(m + 1) * P, :], in_=ot)

    for pool in reversed(ctx_pools):
        pool.release()


_NC_CACHE = {}


def _get_nc():
    if "nc" not in _NC_CACHE:
        _NC_CACHE["nc"] = _build()
    return _NC_CACHE["nc"]


def _host_prep(inputs):
    f32 = lambda a: np.ascontiguousarray(np.asarray(a, dtype=np.float32))
    x = f32(inputs["x"])            # [2, 2048, 768]
    W_qkv = f32(inputs["W_qkv"])    # [768, 2304]
    b_qkv = f32(inputs["b_qkv"])
    W_proj = f32(inputs["W_proj"])
    b_proj = f32(inputs["b_proj"])
    W_fc1 = f32(inputs["W_fc1"])
    b_fc1 = f32(inputs["b_fc1"])
    W_fc2 = f32(inputs["W_fc2"])
    b_fc2 = f32(inputs["b_fc2"])
    ln1_g = f32(inputs["ln1_g"])
    ln1_b = f32(inputs["ln1_b"])
    ln2_g = f32(inputs["ln2_g"])
    ln2_b = f32(inputs["ln2_b"])

    scale = DH ** -0.5
    wqkv_eff = W_qkv * ln1_g[:, None]
    bqkv_eff = ln1_b @ W_qkv + b_qkv
    wqkv_eff[:, :D] *= scale
    bqkv_eff_q = bqkv_eff[:D] * scale
    bqk = np.concatenate([bqkv_eff_q, bqkv_eff[D:2 * D]]).astype(np.float32)
    bv = bqkv_eff[2 * D:]
    bproj_eff = (b_proj + bv @ W_proj).astype(np.float32)
    wfc1_eff = (W_fc1 * ln2_g[:, None]).astype(np.float32)
    bfc1_eff = (ln2_b @ W_fc1 + b_fc1).astype(np.float32)

    bf = lambda a: np.ascontiguousarray(a.astype(ml_dtypes.bfloat16))
    pack = lambda b: np.ascontiguousarray(
        b.reshape(-1, P).T.astype(np.float32))
    shared = {
        "wqkv": bf(wqkv_eff),
        "bqk": pack(bqk),
        "wproj": bf(W_proj),
        "bproj": pack(bproj_eff),
        "wfc1": bf(wfc1_eff),
        "bfc1": pack(bfc1_eff),
        "wfc2": bf(W_fc2),
        "bfc2": pack(b_fc2),
    }
    in_maps = []
    for c in range(N_CORES):
        b, q = divmod(c, 4)
        xb = np.roll(x[b], -TO * q, axis=0)  # own tokens at rows 0:TO
        m = dict(shared)
        m["x_fm"] = bf(xb.T)
        m["x_own"] = np.ascontiguousarray(xb[:TO].T)
        in_maps.append(m)
    return in_maps


def _run(inputs, trace=False):
    nc = _get_nc()
    in_maps = _host_prep(inputs)
    res = bass_utils.run_bass_kernel_spmd(nc, in_maps, list(range(N_CORES)),
                                          trace=trace)
    B = 2
    out = np.empty((B, TB, D), dtype=np.float32)
    for c in range(N_CORES):
        b, q = divmod(c, 4)
        out[b, TO * q:TO * (q + 1), :] = res.results[c]["out_fm"].T
    return out, res


def kernel(**inputs):
    out, _ = _run(inputs, trace=False)
    return out


if __name__ == "__main__":
    print("building...")
    _get_nc()
    print("built ok")


# revision 19
# speedup vs baseline: 1.0251x; 1.0119x over previous
"""Trainium2 Bass kernel for a dense transformer block (pre-LN, MHA + GELU MLP).

Sharding: 8 cores = 2 batches x 4 sequence-quarters. Each core recomputes
LN1 + K/V for its full batch (zero cross-core communication), and computes
Q/attention/proj/MLP for its own 512 tokens only.

Device works feature-major ([feature, token]); the host pre-transposes x and
post-transposes the output. LN gains/biases are folded into the following
matmul weights on the host; the qk scale (1/8) is folded into W_q; the v bias
is folded into b_proj.

Numerics: matmul operands are bf16 (fp32 PSUM accumulation); the residual
stream (x, x2, out), layernorm statistics, and softmax denominators stay fp32.
LN-statistic / broadcast matmuls run in fp32r.
"""
import sys

sys.path.insert(0, "/opt/trn_rl_repo")

import numpy as np
import ml_dtypes

import concourse.bass as bass  # noqa: F401
import concourse.tile as tile
from concourse import bacc, mybir, bass_utils

F32 = mybir.dt.float32
F32R = mybir.dt.float32r
BF16 = mybir.dt.bfloat16
AF = mybir.ActivationFunctionType
ALU = mybir.AluOpType

P = 128
D = 768
NH = 12
DH = 64
DFF = 3072
TB = 2048      # tokens per batch
TO = 512       # tokens owned per core
NJ = D // P    # 6 feature tiles
NT = TB // TO  # 4 token tiles per batch
NTK = TB // P  # 16 key tiles
NMLP = DFF // P  # 24
EPS = 1e-6
N_CORES = 8
VW = 66        # 64 v cols + 2 ones cols per head


def R(ap):
    return ap.bitcast(F32R)


def _build():
    nc = bacc.Bacc("TRN2", target_bir_lowering=False, debug=False,
                   num_devices=N_CORES)

    x_fm = nc.dram_tensor("x_fm", [D, TB], BF16, kind="ExternalInput").ap()
    x_own_d = nc.dram_tensor("x_own", [D, TO], F32, kind="ExternalInput").ap()
    wqkv = nc.dram_tensor("wqkv", [D, 3 * D], BF16, kind="ExternalInput").ap()
    bqk = nc.dram_tensor("bqk", [P, 12], F32, kind="ExternalInput").ap()
    wproj = nc.dram_tensor("wproj", [D, D], BF16, kind="ExternalInput").ap()
    bproj = nc.dram_tensor("bproj", [P, NJ], F32, kind="ExternalInput").ap()
    wfc1 = nc.dram_tensor("wfc1", [D, DFF], BF16, kind="ExternalInput").ap()
    bfc1 = nc.dram_tensor("bfc1", [P, NMLP], F32, kind="ExternalInput").ap()
    wfc2 = nc.dram_tensor("wfc2", [DFF, D], BF16, kind="ExternalInput").ap()
    bfc2 = nc.dram_tensor("bfc2", [P, NJ], F32, kind="ExternalInput").ap()
    out_fm = nc.dram_tensor("out_fm", [D, TO], F32, kind="ExternalOutput").ap()

    with nc.allow_low_precision(reason="bf16 matmul operands are intentional"), \
            tile.TileContext(nc) as tc:
        _emit(tc, nc, x_fm, x_own_d, wqkv, bqk, wproj, bproj, wfc1, bfc1,
              wfc2, bfc2, out_fm)
    nc.compile()
    return nc


def _ln_batched(nc, pools, x_tiles_by_nt, xn_out_fn, ones_t, half2, eps2,
                n_nt, bf16_in):
    """LayerNorm over n_nt token tiles of 512, one batched stats chain.

    x_tiles_by_nt[nt][j]: input tiles [128, 512] (bf16 if bf16_in else fp32).
    xn_out_fn(nt, j) -> bf16 dest AP [128, 512].
    """
    tc, stats, sq_pool, ln_ps, bc_ps = pools
    ntot = 512 * n_nt
    sum_sb = stats.tile([2, ntot], F32, tag="sum_sb", name="sum_sb")
    sq_sb = stats.tile([2, ntot], F32, tag="sq_sb", name="sq_sb")
    mk = (lambda ap: ap) if bf16_in else R
    sqdt = BF16 if bf16_in else F32
    for nt in range(n_nt):
        x_tiles = x_tiles_by_nt[nt]
        xsq = []
        for j in range(NJ):
            t = sq_pool.tile([P, 512], sqdt, tag=f"xsq{j}", name="xsqt")
            nc.scalar.activation(out=mk(t), in_=x_tiles[j], func=AF.Square)
            xsq.append(t)
        ps_sum = ln_ps.tile([2, 512], F32, tag="lnsum", name="ps_sum")
        ps_sq = ln_ps.tile([2, 512], F32, tag="lnsq", name="ps_sq")
        for j in range(NJ):
            nc.tensor.matmul(ps_sum[:], lhsT=mk(ones_t), rhs=mk(x_tiles[j]),
                             start=(j == 0), stop=(j == NJ - 1))
        for j in range(NJ):
            nc.tensor.matmul(ps_sq[:], lhsT=mk(ones_t), rhs=mk(xsq[j]),
                             start=(j == 0), stop=(j == NJ - 1))
        sl = slice(nt * 512, (nt + 1) * 512)
        nc.vector.tensor_copy(out=sum_sb[:, sl], in_=ps_sum)
        nc.vector.tensor_copy(out=sq_sb[:, sl], in_=ps_sq)
    # var*D^2 = D*sumsq - sum^2 ; rs = exp(-0.5*ln(varD2/D^2 + eps))
    t1 = stats.tile([2, ntot], F32, tag="t1", name="t1")
    nc.vector.scalar_tensor_tensor(out=t1, in0=sum_sb, scalar=-1.0,
                                   in1=sum_sb, op0=ALU.mult, op1=ALU.mult)
    nc.vector.scalar_tensor_tensor(out=t1, in0=sq_sb, scalar=float(D),
                                   in1=t1, op0=ALU.mult, op1=ALU.add)
    nc.scalar.activation(out=t1, in_=t1, func=AF.Ln, bias=eps2,
                         scale=1.0 / (D * D))
    rs = stats.tile([2, ntot], F32, tag="rs", name="rs")
    nc.scalar.activation(out=R(rs), in_=t1, func=AF.Exp, scale=-0.5)
    cc = stats.tile([2, ntot], F32, tag="cc", name="cc")
    nc.vector.scalar_tensor_tensor(out=R(cc), in0=sum_sb, scalar=-1.0 / D,
                                   in1=rs, op0=ALU.mult, op1=ALU.mult)
    for nt in range(n_nt):
        sl = slice(nt * 512, (nt + 1) * 512)
        ps_a = bc_ps.tile([P, 512], F32, tag="bca", name="ps_a")
        nc.tensor.matmul(ps_a[:], lhsT=R(half2), rhs=R(rs[:, sl]),
                         start=True, stop=True)
        ps_c = bc_ps.tile([P, 512], F32, tag="bcc", name="ps_c")
        nc.tensor.matmul(ps_c[:], lhsT=R(half2), rhs=R(cc[:, sl]),
                         start=True, stop=True)
        for j in range(NJ):
            tmp = sq_pool.tile([P, 512], F32, tag=f"tmp{j}", name="xnt")
            nc.vector.tensor_mul(out=tmp, in0=x_tiles_by_nt[nt][j], in1=ps_a)
            nc.vector.tensor_add(out=xn_out_fn(nt, j), in0=tmp, in1=ps_c)


def _emit(tc, nc, x_fm, x_own_d, wqkv, bqk, wproj_d, bproj_d, wfc1_d, bfc1_d,
          wfc2_d, bfc2_d, out_fm):
    ctx_pools = []

    cons_pool = tc.alloc_tile_pool(name="cons", bufs=1)
    ctx_pools.append(cons_pool)
    ones2 = cons_pool.tile([P, 2], F32)
    nc.vector.memset(ones2, 1.0)
    ones2b = cons_pool.tile([P, 2], BF16)
    nc.vector.memset(ones2b, 1.0)
    half2 = cons_pool.tile([2, P], F32)
    nc.vector.memset(half2, 0.5)
    eps2 = cons_pool.tile([2, 1], F32)
    nc.vector.memset(eps2, EPS)
    e0h = cons_pool.tile([2, 64], F32)
    nc.vector.memset(e0h, 0.0)
    nc.vector.memset(e0h[0:1, :], 1.0)
    e1h = cons_pool.tile([2, 64], F32)
    nc.vector.memset(e1h, 1.0)
    nc.vector.memset(e1h[0:1, :], 0.0)

    bqk_sb = cons_pool.tile([P, 12], F32)
    nc.sync.dma_start(out=bqk_sb, in_=bqk)
    bproj_sb = cons_pool.tile([P, NJ], F32)
    nc.sync.dma_start(out=bproj_sb, in_=bproj_d)
    bfc1_sb = cons_pool.tile([P, NMLP], F32)
    nc.sync.dma_start(out=bfc1_sb, in_=bfc1_d)
    bfc2_sb = cons_pool.tile([P, NJ], F32)
    nc.sync.dma_start(out=bfc2_sb, in_=bfc2_d)

    stats = tc.alloc_tile_pool(name="stats", bufs=2)
    ctx_pools.append(stats)

    # k/q bf16; x_own fp32 residual; live until proj.
    persist = tc.alloc_tile_pool(name="persist", bufs=1)
    k_sb = [persist.tile([P, TB], BF16, tag=f"k{j}", name=f"k{j}")
            for j in range(NJ)]
    q_sb = [persist.tile([P, TO], BF16, tag=f"q{j}", name=f"q{j}")
            for j in range(NJ)]
    x_own = [persist.tile([P, TO], F32, tag=f"xo{j}", name=f"xo{j}")
             for j in range(NJ)]

    v_pool = tc.alloc_tile_pool(name="vpool", bufs=1, side="right")
    v_sb = [v_pool.tile([P, NH * VW], BF16, tag=f"v{t}", name=f"v{t}")
            for t in range(NTK)]

    xn_pool = tc.alloc_tile_pool(name="xnpool", bufs=1)
    xn_all = [xn_pool.tile([P, TB], BF16, tag=f"xn{j}", name=f"xn{j}")
              for j in range(NJ)]

    # ---------------- Phase 1: load x (bf16), LN1 -> xn_all (bf16) ---------
    with (
        tc.tile_pool(name="xstream", bufs=1) as xpool,
        tc.tile_pool(name="sqpool", bufs=2) as sq_pool,
        tc.tile_pool(name="lnps", bufs=2, space="PSUM") as ln_ps,
        tc.tile_pool(name="bcps", bufs=2, space="PSUM") as bc_ps,
    ):
        for j in range(NJ):
            nc.sync.dma_start(out=x_own[j], in_=x_own_d[j * P:(j + 1) * P, :])
        pools = (tc, stats, sq_pool, ln_ps, bc_ps)
        for nt in range(NT):
            xt = [xpool.tile([P, TO], BF16, tag=f"xs{nt}_{j}",
                             name=f"xs{nt}_{j}") for j in range(NJ)]
            for j in range(NJ):
                nc.sync.dma_start(
                    out=xt[j],
                    in_=x_fm[j * P:(j + 1) * P, nt * TO:(nt + 1) * TO])
            _ln_batched(nc, pools, [xt],
                        lambda n_, j, nt=nt: xn_all[j][:, nt * TO:(nt + 1) * TO],
                        ones2b, half2, eps2, 1, True)

    # ---------------- Phase 2: Q, V, then K (bf16) ----------------
    # V is emitted before K so that attention (gated on K) starts only after
    # V is resident; av accumulation then never convoys behind the V matmuls.
    with (
        tc.tile_pool(name="wkq", bufs=1) as wkq_pool,
        tc.tile_pool(name="mmps", bufs=4, space="PSUM") as mm_ps,
        tc.tile_pool(name="wv", bufs=1) as wv_pool,
        tc.tile_pool(name="vps5", bufs=2, space="PSUM") as v_ps5,
    ):
        wkq = []
        for j in range(NJ):
            t = wkq_pool.tile([P, 2 * D], BF16, tag=f"wkq{j}", name=f"wkq{j}")
            nc.sync.dma_start(out=t, in_=wqkv[j * P:(j + 1) * P, 0:2 * D])
            wkq.append(t)
        wv = []
        for j in range(NJ):
            t = wv_pool.tile([P, D], BF16, tag=f"wv{j}", name=f"wv{j}")
            nc.sync.dma_start(out=t, in_=wqkv[j * P:(j + 1) * P, 2 * D:3 * D])
            wv.append(t)
        # Q for own tokens
        for m in range(NJ):
            pt = mm_ps.tile([P, TO], F32, tag="mm", name="mmq")
            for j in range(NJ):
                nc.tensor.matmul(pt[:], lhsT=wkq[j][:, m * P:(m + 1) * P],
                                 rhs=xn_all[j][:, 0:TO],
                                 start=(j == 0), stop=(j == NJ - 1))
            nc.vector.tensor_scalar_add(q_sb[m], pt, bqk_sb[:, m:m + 1])
        # V (token-major with ones columns)
        for mt in range(NTK):
            vt = v_sb[mt]
            nc.vector.memset(
                vt.rearrange("p (h w) -> p h w", w=VW)[:, :, 64:66], 1.0)
            pt5 = v_ps5.tile([P, 512], F32, tag="v5", name="v5")
            pt2 = mm_ps.tile([P, TO], F32, tag="mm", name="v2")
            for j in range(NJ):
                lhs = xn_all[j][:, mt * P:(mt + 1) * P]
                nc.tensor.matmul(pt5[:], lhsT=lhs, rhs=wv[j][:, 0:512],
                                 start=(j == 0), stop=(j == NJ - 1))
            for j in range(NJ):
                lhs = xn_all[j][:, mt * P:(mt + 1) * P]
                nc.tensor.matmul(pt2[:, 0:256], lhsT=lhs, rhs=wv[j][:, 512:768],
                                 start=(j == 0), stop=(j == NJ - 1))
            v3 = vt.rearrange("p (h w) -> p h w", w=VW)
            nc.vector.tensor_copy(
                out=v3[:, 0:8, 0:64],
                in_=pt5.rearrange("p (h w) -> p h w", w=64))
            nc.vector.tensor_copy(
                out=v3[:, 8:12, 0:64],
                in_=pt2[:, 0:256].rearrange("p (h w) -> p h w", w=64))
        # K for all tokens
        for m in range(NJ):
            for nt in range(NT):
                pt = mm_ps.tile([P, TO], F32, tag="mm", name="mmk")
                for j in range(NJ):
                    nc.tensor.matmul(
                        pt[:], lhsT=wkq[j][:, D + m * P:D + (m + 1) * P],
                        rhs=xn_all[j][:, nt * TO:(nt + 1) * TO],
                        start=(j == 0), stop=(j == NJ - 1))
                nc.vector.tensor_scalar_add(
                    k_sb[m][:, nt * TO:(nt + 1) * TO], pt,
                    bqk_sb[:, 6 + m:7 + m])
    xn_pool.release()

    # ---------------- Phase 4: attention ----------------
    attn_pool = tc.alloc_tile_pool(name="attnpool", bufs=1)
    attn_fm = [attn_pool.tile([P, TO], BF16, tag=f"at{j}", name=f"at{j}")
               for j in range(NJ)]
    wp_pool = tc.alloc_tile_pool(name="wproj", bufs=1)
    wp = []
    for j in range(NJ):
        t = wp_pool.tile([P, D], BF16, tag=f"wp{j}", name=f"wp{j}")
        nc.sync.dma_start(out=t, in_=wproj_d[j * P:(j + 1) * P, :])
        wp.append(t)
    with (
        tc.tile_pool(name="seps", bufs=3, space="PSUM") as se_ps,
        tc.tile_pool(name="avps", bufs=1, space="PSUM") as av_ps,
        tc.tile_pool(name="sesb", bufs=6) as se_pool,
        tc.tile_pool(name="bcsb", bufs=2) as bc_pool,
    ):
        for hp in range(NJ):
            pt_av_a = av_ps.tile([P, 512], F32, tag="ava", name="ava")
            pt_av_b = av_ps.tile([P, 512], F32, tag="avb", name="avb")
            for tk2 in range(NTK // 2):
                ps_a = se_ps.tile([P, 1024], F32, tag="se", name="psea")
                ps_b = se_ps.tile([P, 1024], F32, tag="se", name="pseb")
                for half in range(2):
                    tk = 2 * tk2 + half
                    ksl = slice(tk * P, (tk + 1) * P)
                    fsl = slice(half * 512, (half + 1) * 512)
                    nc.tensor.matmul(ps_a[:, fsl],
                                     lhsT=k_sb[hp][0:64, ksl],
                                     rhs=q_sb[hp][0:64, :],
                                     start=True, stop=True)
                    nc.tensor.matmul(ps_b[:, fsl],
                                     lhsT=k_sb[hp][64:128, ksl],
                                     rhs=q_sb[hp][64:128, :],
                                     start=True, stop=True)
                se_a = se_pool.tile([P, 1024], BF16, tag="sea", name="sea")
                se_b = se_pool.tile([P, 1024], BF16, tag="seb", name="seb")
                nc.scalar.activation(out=se_a, in_=ps_a, func=AF.Exp)
                nc.scalar.activation(out=se_b, in_=ps_b, func=AF.Exp)
                for half in range(2):
                    tk = 2 * tk2 + half
                    fsl = slice(half * 512, (half + 1) * 512)
                    first = (tk == 0)
                    last = (tk == NTK - 1)
                    nc.tensor.matmul(
                        pt_av_a[:VW, :],
                        lhsT=v_sb[tk][:, (2 * hp) * VW:(2 * hp + 1) * VW],
                        rhs=se_a[:, fsl], start=first, stop=last)
                    nc.tensor.matmul(
                        pt_av_b[:VW, :],
                        lhsT=v_sb[tk][:, (2 * hp + 1) * VW:(2 * hp + 2) * VW],
                        rhs=se_b[:, fsl], start=first, stop=last)
            for head, pt_av in ((0, pt_av_a), (1, pt_av_b)):
                # DVE reciprocal: DVE is idle during the ACT-bound exp stream
                rec = bc_pool.tile([2, 512], F32, tag="rec", name="rec")
                nc.vector.reciprocal(out=R(rec), in_=pt_av[64:66, :])
                ps_bc = se_ps.tile([64, 512], F32, tag="se", name="psbc")
                nc.tensor.matmul(ps_bc[:], lhsT=R(half2[:, 0:64]), rhs=R(rec),
                                 start=True, stop=True)
                bc_sb = bc_pool.tile([64, 512], F32, tag="bc", name="bcsb")
                nc.vector.tensor_copy(out=bc_sb, in_=ps_bc)
                nc.vector.tensor_mul(
                    out=attn_fm[hp][head * 64:(head + 1) * 64, :],
                    in0=pt_av[0:64, :], in1=bc_sb)
    v_pool.release()

    # ---------------- Phase 5: proj + residual -> x2 (fp32) ----------------
    x2_pool = tc.alloc_tile_pool(name="x2pool", bufs=1, side="right")
    ctx_pools.append(x2_pool)
    x2_sb = [x2_pool.tile([P, TO], F32, tag=f"x2{j}", name=f"x2{j}")
             for j in range(NJ)]
    wfc1_pool = tc.alloc_tile_pool(name="wfc1", bufs=1, side="right")
    ctx_pools.append(wfc1_pool)
    wf1 = []
    for j in range(NJ):
        t = wfc1_pool.tile([P, DFF], BF16, tag=f"wf1{j}", name=f"wf1{j}")
        nc.sync.dma_start(out=t, in_=wfc1_d[j * P:(j + 1) * P, :])
        wf1.append(t)
    with (
        tc.tile_pool(name="mmps2", bufs=3, space="PSUM") as mm_ps2,
    ):
        for m in range(NJ):
            pt = mm_ps2.tile([P, TO], F32, tag="mm", name="mmproj")
            for j in range(NJ):
                nc.tensor.matmul(pt[:], lhsT=wp[j][:, m * P:(m + 1) * P],
                                 rhs=attn_fm[j],
                                 start=(j == 0), stop=(j == NJ - 1))
            nc.vector.scalar_tensor_tensor(
                out=R(x2_sb[m]), in0=pt, scalar=bproj_sb[:, m:m + 1],
                in1=x_own[m], op0=ALU.add, op1=ALU.add)
    wp_pool.release()
    attn_pool.release()
    persist.release()

    # ---------------- Phase 6: LN2 -> h (bf16) ----------------
    h_pool = tc.alloc_tile_pool(name="hpool", bufs=1, side="right")
    ctx_pools.append(h_pool)
    h_sb = [h_pool.tile([P, TO], BF16, tag=f"h{j}", name=f"h{j}")
            for j in range(NJ)]
    with (
        tc.tile_pool(name="sqpool2", bufs=2) as sq_pool2,
        tc.tile_pool(name="lnps2", bufs=1, space="PSUM") as ln_ps2,
        tc.tile_pool(name="bcps3", bufs=1, space="PSUM") as bc_ps3,
    ):
        pools = (tc, stats, sq_pool2, ln_ps2, bc_ps3)
        _ln_batched(nc, pools, [x2_sb],
                    lambda nt, j: h_sb[j][:, :],
                    ones2, half2, eps2, 1, False)

    # ---------------- Phase 7: fc1 + gelu -> h1 (bf16) ----------------
    h1_pool = tc.alloc_tile_pool(name="h1", bufs=1, side="right")
    ctx_pools.append(h1_pool)
    h1_sb = [h1_pool.tile([P, TO], BF16, tag=f"h1{m}", name=f"h1{m}")
             for m in range(NMLP)]
    with (
        tc.tile_pool(name="mmps3", bufs=4, space="PSUM") as mm_ps3,
    ):
        for m in range(NMLP):
            pt = mm_ps3.tile([P, TO], F32, tag="mm", name="mmfc1")
            for j in range(NJ):
                nc.tensor.matmul(pt[:], lhsT=wf1[j][:, m * P:(m + 1) * P],
                                 rhs=h_sb[j],
                                 start=(j == 0), stop=(j == NJ - 1))
            nc.scalar.activation(out=h1_sb[m], in_=pt, func=AF.Gelu,
                                 bias=bfc1_sb[:, m:m + 1])

    # ---------------- Phase 8: fc2 + residual + store ----------------
    with (
        tc.tile_pool(name="wfc2", bufs=6) as wfc2_pool,
        tc.tile_pool(name="fc2ps", bufs=1, space="PSUM") as fc2_ps,
        tc.tile_pool(name="outsb", bufs=2) as out_pool,
    ):
        pts = [fc2_ps.tile([P, TO], F32, tag=f"fc2_{m}", name=f"fc2_{m}")
               for m in range(NJ)]
        for j in range(NMLP):
            wt = wfc2_pool.tile([P, D], BF16, tag="wf2", name="wf2")
            nc.sync.dma_start(out=wt, in_=wfc2_d[j * P:(j + 1) * P, :])
            for m in range(NJ):
                nc.tensor.matmul(pts[m][:], lhsT=wt[:, m * P:(m + 1) * P],
                                 rhs=h1_sb[j],
                                 start=(j == 0), stop=(j == NMLP - 1))
        for m in range(NJ):
            ot = out_pool.tile([P, TO], F32, tag="out", name="ot")
            nc.vector.scalar_tensor_tensor(
                out=ot, in0=pts[m], scalar=bfc2_sb[:, m:m + 1],
                in1=x2_sb[m], op0=ALU.add, op1=ALU.add)
            nc.sync.dma_start(out=out_fm[m * P:(m + 1) * P, :], in_=ot)

    for pool in reversed(ctx_pools):
        pool.release()


_NC_CACHE = {}


def _get_nc():
    if "nc" not in _NC_CACHE:
        _NC_CACHE["nc"] = _build()
    return _NC_CACHE["nc"]


def _host_prep(inputs):
    f32 = lambda a: np.ascontiguousarray(np.asarray(a, dtype=np.float32))
    x = f32(inputs["x"])            # [2, 2048, 768]
    W_qkv = f32(inputs["W_qkv"])    # [768, 2304]
    b_qkv = f32(inputs["b_qkv"])
    W_proj = f32(inputs["W_proj"])
    b_proj = f32(inputs["b_proj"])
    W_fc1 = f32(inputs["W_fc1"])
    b_fc1 = f32(inputs["b_fc1"])
    W_fc2 = f32(inputs["W_fc2"])
    b_fc2 = f32(inputs["b_fc2"])
    ln1_g = f32(inputs["ln1_g"])
    ln1_b = f32(inputs["ln1_b"])
    ln2_g = f32(inputs["ln2_g"])
    ln2_b = f32(inputs["ln2_b"])

    scale = DH ** -0.5
    wqkv_eff = W_qkv * ln1_g[:, None]
    bqkv_eff = ln1_b @ W_qkv + b_qkv
    wqkv_eff[:, :D] *= scale
    bqkv_eff_q = bqkv_eff[:D] * scale
    bqk = np.concatenate([bqkv_eff_q, bqkv_eff[D:2 * D]]).astype(np.float32)
    bv = bqkv_eff[2 * D:]
    bproj_eff = (b_proj + bv @ W_proj).astype(np.float32)
    wfc1_eff = (W_fc1 * ln2_g[:, None]).astype(np.float32)
    bfc1_eff = (ln2_b @ W_fc1 + b_fc1).astype(np.float32)

    bf = lambda a: np.ascontiguousarray(a.astype(ml_dtypes.bfloat16))
    pack = lambda b: np.ascontiguousarray(
        b.reshape(-1, P).T.astype(np.float32))
    shared = {
        "wqkv": bf(wqkv_eff),
        "bqk": pack(bqk),
        "wproj": bf(W_proj),
        "bproj": pack(bproj_eff),
        "wfc1": bf(wfc1_eff),
        "bfc1": pack(bfc1_eff),
        "wfc2": bf(W_fc2),
        "bfc2": pack(b_fc2),
    }
    in_maps = []
    for c in range(N_CORES):
        b, q = divmod(c, 4)
        xb = np.roll(x[b], -TO * q, axis=0)  # own tokens at rows 0:TO
        m = dict(shared)
        m["x_fm"] = bf(xb.T)
        m["x_own"] = np.ascontiguousarray(xb[:TO].T)
        in_maps.append(m)
    return in_maps


def _run(inputs, trace=False):
    nc = _get_nc()
    in_maps = _host_prep(inputs)
    res = bass_utils.run_bass_kernel_spmd(nc, in_maps, list(range(N_CORES)),
                                          trace=trace)
    B = 2
    out = np.empty((B, TB, D), dtype=np.float32)
    for c in range(N_CORES):
        b, q = divmod(c, 4)
        out[b, TO * q:TO * (q + 1), :] = res.results[c]["out_fm"].T
    return out, res


def kernel(**inputs):
    out, _ = _run(inputs, trace=False)
    return out


if __name__ == "__main__":
    print("building...")
    _get_nc()
    print("built ok")


# revision 20
# speedup vs baseline: 1.0276x; 1.0024x over previous
"""Trainium2 Bass kernel for a dense transformer block (pre-LN, MHA + GELU MLP).

Sharding: 8 cores = 2 batches x 4 sequence-quarters. Each core recomputes
LN1 + K/V for its full batch (zero cross-core communication), and computes
Q/attention/proj/MLP for its own 512 tokens only.

Device works feature-major ([feature, token]); the host pre-transposes x and
post-transposes the output. LN gains/biases are folded into the following
matmul weights on the host; the qk scale (1/8) is folded into W_q; the v bias
is folded into b_proj.

Numerics: matmul operands are bf16 (fp32 PSUM accumulation); the residual
stream (x, x2, out), layernorm statistics, and softmax denominators stay fp32.
LN-statistic / broadcast matmuls run in fp32r.
"""
import sys

sys.path.insert(0, "/opt/trn_rl_repo")

import numpy as np
import ml_dtypes

import concourse.bass as bass  # noqa: F401
import concourse.tile as tile
from concourse import bacc, mybir, bass_utils

F32 = mybir.dt.float32
F32R = mybir.dt.float32r
BF16 = mybir.dt.bfloat16
AF = mybir.ActivationFunctionType
ALU = mybir.AluOpType

P = 128
D = 768
NH = 12
DH = 64
DFF = 3072
TB = 2048      # tokens per batch
TO = 512       # tokens owned per core
NJ = D // P    # 6 feature tiles
NT = TB // TO  # 4 token tiles per batch
NTK = TB // P  # 16 key tiles
NMLP = DFF // P  # 24
EPS = 1e-6
N_CORES = 8
VW = 66        # 64 v cols + 2 ones cols per head


def R(ap):
    return ap.bitcast(F32R)


def _build():
    nc = bacc.Bacc("TRN2", target_bir_lowering=False, debug=False,
                   num_devices=N_CORES)

    x_fm = nc.dram_tensor("x_fm", [D, TB], BF16, kind="ExternalInput").ap()
    x_own_d = nc.dram_tensor("x_own", [D, TO], F32, kind="ExternalInput").ap()
    wqkv = nc.dram_tensor("wqkv", [D, 3 * D], BF16, kind="ExternalInput").ap()
    bqk = nc.dram_tensor("bqk", [P, 12], F32, kind="ExternalInput").ap()
    wproj = nc.dram_tensor("wproj", [D, D], BF16, kind="ExternalInput").ap()
    bproj = nc.dram_tensor("bproj", [P, NJ], F32, kind="ExternalInput").ap()
    wfc1 = nc.dram_tensor("wfc1", [D, DFF], BF16, kind="ExternalInput").ap()
    bfc1 = nc.dram_tensor("bfc1", [P, NMLP], F32, kind="ExternalInput").ap()
    wfc2 = nc.dram_tensor("wfc2", [DFF, D], BF16, kind="ExternalInput").ap()
    bfc2 = nc.dram_tensor("bfc2", [P, NJ], F32, kind="ExternalInput").ap()
    out_fm = nc.dram_tensor("out_fm", [D, TO], F32, kind="ExternalOutput").ap()

    with nc.allow_low_precision(reason="bf16 matmul operands are intentional"), \
            tile.TileContext(nc) as tc:
        _emit(tc, nc, x_fm, x_own_d, wqkv, bqk, wproj, bproj, wfc1, bfc1,
              wfc2, bfc2, out_fm)
    nc.compile()
    return nc


def _ln_batched(nc, pools, x_tiles_by_nt, xn_out_fn, ones_t, half2, eps2,
                n_nt, bf16_in):
    """LayerNorm over n_nt token tiles of 512, one batched stats chain.

    x_tiles_by_nt[nt][j]: input tiles [128, 512] (bf16 if bf16_in else fp32).
    xn_out_fn(nt, j) -> bf16 dest AP [128, 512].
    """
    tc, stats, sq_pool, ln_ps, bc_ps = pools
    ntot = 512 * n_nt
    sum_sb = stats.tile([2, ntot], F32, tag="sum_sb", name="sum_sb")
    sq_sb = stats.tile([2, ntot], F32, tag="sq_sb", name="sq_sb")
    mk = (lambda ap: ap) if bf16_in else R
    sqdt = BF16 if bf16_in else F32
    for nt in range(n_nt):
        x_tiles = x_tiles_by_nt[nt]
        xsq = []
        for j in range(NJ):
            t = sq_pool.tile([P, 512], sqdt, tag=f"xsq{j}", name="xsqt")
            nc.scalar.activation(out=mk(t), in_=x_tiles[j], func=AF.Square)
            xsq.append(t)
        ps_sum = ln_ps.tile([2, 512], F32, tag="lnsum", name="ps_sum")
        ps_sq = ln_ps.tile([2, 512], F32, tag="lnsq", name="ps_sq")
        for j in range(NJ):
            nc.tensor.matmul(ps_sum[:], lhsT=mk(ones_t), rhs=mk(x_tiles[j]),
                             start=(j == 0), stop=(j == NJ - 1))
        for j in range(NJ):
            nc.tensor.matmul(ps_sq[:], lhsT=mk(ones_t), rhs=mk(xsq[j]),
                             start=(j == 0), stop=(j == NJ - 1))
        sl = slice(nt * 512, (nt + 1) * 512)
        nc.vector.tensor_copy(out=sum_sb[:, sl], in_=ps_sum)
        nc.vector.tensor_copy(out=sq_sb[:, sl], in_=ps_sq)
    # var*D^2 = D*sumsq - sum^2 ; rs = exp(-0.5*ln(varD2/D^2 + eps))
    t1 = stats.tile([2, ntot], F32, tag="t1", name="t1")
    nc.vector.scalar_tensor_tensor(out=t1, in0=sum_sb, scalar=-1.0,
                                   in1=sum_sb, op0=ALU.mult, op1=ALU.mult)
    nc.vector.scalar_tensor_tensor(out=t1, in0=sq_sb, scalar=float(D),
                                   in1=t1, op0=ALU.mult, op1=ALU.add)
    nc.scalar.activation(out=t1, in_=t1, func=AF.Ln, bias=eps2,
                         scale=1.0 / (D * D))
    rs = stats.tile([2, ntot], F32, tag="rs", name="rs")
    nc.scalar.activation(out=R(rs), in_=t1, func=AF.Exp, scale=-0.5)
    cc = stats.tile([2, ntot], F32, tag="cc", name="cc")
    nc.vector.scalar_tensor_tensor(out=R(cc), in0=sum_sb, scalar=-1.0 / D,
                                   in1=rs, op0=ALU.mult, op1=ALU.mult)
    for nt in range(n_nt):
        sl = slice(nt * 512, (nt + 1) * 512)
        ps_a = bc_ps.tile([P, 512], F32, tag="bca", name="ps_a")
        nc.tensor.matmul(ps_a[:], lhsT=R(half2), rhs=R(rs[:, sl]),
                         start=True, stop=True)
        ps_c = bc_ps.tile([P, 512], F32, tag="bcc", name="ps_c")
        nc.tensor.matmul(ps_c[:], lhsT=R(half2), rhs=R(cc[:, sl]),
                         start=True, stop=True)
        for j in range(NJ):
            tmp = sq_pool.tile([P, 512], F32, tag=f"tmp{j}", name="xnt")
            nc.vector.tensor_mul(out=tmp, in0=x_tiles_by_nt[nt][j], in1=ps_a)
            nc.vector.tensor_add(out=xn_out_fn(nt, j), in0=tmp, in1=ps_c)


def _emit(tc, nc, x_fm, x_own_d, wqkv, bqk, wproj_d, bproj_d, wfc1_d, bfc1_d,
          wfc2_d, bfc2_d, out_fm):
    ctx_pools = []

    cons_pool = tc.alloc_tile_pool(name="cons", bufs=1)
    ctx_pools.append(cons_pool)
    ones2 = cons_pool.tile([P, 2], F32)
    nc.vector.memset(ones2, 1.0)
    ones2b = cons_pool.tile([P, 2], BF16)
    nc.vector.memset(ones2b, 1.0)
    half2 = cons_pool.tile([2, P], F32)
    nc.vector.memset(half2, 0.5)
    eps2 = cons_pool.tile([2, 1], F32)
    nc.vector.memset(eps2, EPS)
    e0h = cons_pool.tile([2, 64], F32)
    nc.vector.memset(e0h, 0.0)
    nc.vector.memset(e0h[0:1, :], 1.0)
    e1h = cons_pool.tile([2, 64], F32)
    nc.vector.memset(e1h, 1.0)
    nc.vector.memset(e1h[0:1, :], 0.0)

    bqk_sb = cons_pool.tile([P, 12], F32)
    nc.sync.dma_start(out=bqk_sb, in_=bqk)
    bproj_sb = cons_pool.tile([P, NJ], F32)
    nc.sync.dma_start(out=bproj_sb, in_=bproj_d)
    bfc1_sb = cons_pool.tile([P, NMLP], F32)
    nc.sync.dma_start(out=bfc1_sb, in_=bfc1_d)
    bfc2_sb = cons_pool.tile([P, NJ], F32)
    nc.sync.dma_start(out=bfc2_sb, in_=bfc2_d)

    stats = tc.alloc_tile_pool(name="stats", bufs=2)
    ctx_pools.append(stats)

    # k/q bf16; x_own fp32 residual; live until proj.
    persist = tc.alloc_tile_pool(name="persist", bufs=1)
    k_sb = [persist.tile([P, TB], BF16, tag=f"k{j}", name=f"k{j}")
            for j in range(NJ)]
    q_sb = [persist.tile([P, TO], BF16, tag=f"q{j}", name=f"q{j}")
            for j in range(NJ)]
    x_own = [persist.tile([P, TO], F32, tag=f"xo{j}", name=f"xo{j}")
             for j in range(NJ)]

    v_pool = tc.alloc_tile_pool(name="vpool", bufs=1, side="right")
    v_sb = [v_pool.tile([P, NH * VW], BF16, tag=f"v{t}", name=f"v{t}")
            for t in range(NTK)]

    xn_pool = tc.alloc_tile_pool(name="xnpool", bufs=1)
    xn_all = [xn_pool.tile([P, TB], BF16, tag=f"xn{j}", name=f"xn{j}")
              for j in range(NJ)]

    # ---------------- Phase 1: load x (bf16), LN1 -> xn_all (bf16) ---------
    with (
        tc.tile_pool(name="xstream", bufs=1) as xpool,
        tc.tile_pool(name="sqpool", bufs=2) as sq_pool,
        tc.tile_pool(name="lnps", bufs=2, space="PSUM") as ln_ps,
        tc.tile_pool(name="bcps", bufs=2, space="PSUM") as bc_ps,
    ):
        for j in range(NJ):
            nc.sync.dma_start(out=x_own[j], in_=x_own_d[j * P:(j + 1) * P, :])
        pools = (tc, stats, sq_pool, ln_ps, bc_ps)
        for nt in range(NT):
            xt = [xpool.tile([P, TO], BF16, tag=f"xs{nt}_{j}",
                             name=f"xs{nt}_{j}") for j in range(NJ)]
            for j in range(NJ):
                nc.sync.dma_start(
                    out=xt[j],
                    in_=x_fm[j * P:(j + 1) * P, nt * TO:(nt + 1) * TO])
            _ln_batched(nc, pools, [xt],
                        lambda n_, j, nt=nt: xn_all[j][:, nt * TO:(nt + 1) * TO],
                        ones2b, half2, eps2, 1, True)

    # ---------------- Phase 2: Q, V, then K (bf16) ----------------
    # V is emitted before K so that attention (gated on K) starts only after
    # V is resident; av accumulation then never convoys behind the V matmuls.
    with (
        tc.tile_pool(name="wkq", bufs=1) as wkq_pool,
        tc.tile_pool(name="mmps", bufs=4, space="PSUM") as mm_ps,
        tc.tile_pool(name="wv", bufs=1) as wv_pool,
        tc.tile_pool(name="vps5", bufs=2, space="PSUM") as v_ps5,
    ):
        wkq = []
        for j in range(NJ):
            t = wkq_pool.tile([P, 2 * D], BF16, tag=f"wkq{j}", name=f"wkq{j}")
            nc.sync.dma_start(out=t, in_=wqkv[j * P:(j + 1) * P, 0:2 * D])
            wkq.append(t)
        wv = []
        for j in range(NJ):
            t = wv_pool.tile([P, D], BF16, tag=f"wv{j}", name=f"wv{j}")
            nc.sync.dma_start(out=t, in_=wqkv[j * P:(j + 1) * P, 2 * D:3 * D])
            wv.append(t)
        # Q for own tokens
        for m in range(NJ):
            pt = mm_ps.tile([P, TO], F32, tag="mm", name="mmq")
            for j in range(NJ):
                nc.tensor.matmul(pt[:], lhsT=wkq[j][:, m * P:(m + 1) * P],
                                 rhs=xn_all[j][:, 0:TO],
                                 start=(j == 0), stop=(j == NJ - 1))
            nc.vector.tensor_scalar_add(q_sb[m], pt, bqk_sb[:, m:m + 1])
        # V (token-major with ones columns)
        for mt in range(NTK):
            vt = v_sb[mt]
            nc.vector.memset(
                vt.rearrange("p (h w) -> p h w", w=VW)[:, :, 64:66], 1.0)
            pt5 = v_ps5.tile([P, 512], F32, tag="v5", name="v5")
            pt2 = mm_ps.tile([P, TO], F32, tag="mm", name="v2")
            for j in range(NJ):
                lhs = xn_all[j][:, mt * P:(mt + 1) * P]
                nc.tensor.matmul(pt5[:], lhsT=lhs, rhs=wv[j][:, 0:512],
                                 start=(j == 0), stop=(j == NJ - 1))
            for j in range(NJ):
                lhs = xn_all[j][:, mt * P:(mt + 1) * P]
                nc.tensor.matmul(pt2[:, 0:256], lhsT=lhs, rhs=wv[j][:, 512:768],
                                 start=(j == 0), stop=(j == NJ - 1))
            v3 = vt.rearrange("p (h w) -> p h w", w=VW)
            nc.vector.tensor_copy(
                out=v3[:, 0:8, 0:64],
                in_=pt5.rearrange("p (h w) -> p h w", w=64))
            nc.vector.tensor_copy(
                out=v3[:, 8:12, 0:64],
                in_=pt2[:, 0:256].rearrange("p (h w) -> p h w", w=64))
        # K for all tokens
        for m in range(NJ):
            for nt in range(NT):
                pt = mm_ps.tile([P, TO], F32, tag="mm", name="mmk")
                for j in range(NJ):
                    nc.tensor.matmul(
                        pt[:], lhsT=wkq[j][:, D + m * P:D + (m + 1) * P],
                        rhs=xn_all[j][:, nt * TO:(nt + 1) * TO],
                        start=(j == 0), stop=(j == NJ - 1))
                nc.vector.tensor_scalar_add(
                    k_sb[m][:, nt * TO:(nt + 1) * TO], pt,
                    bqk_sb[:, 6 + m:7 + m])
    xn_pool.release()

    # ---------------- Phase 4: attention ----------------
    attn_pool = tc.alloc_tile_pool(name="attnpool", bufs=1)
    attn_fm = [attn_pool.tile([P, TO], BF16, tag=f"at{j}", name=f"at{j}")
               for j in range(NJ)]
    av_sb = [attn_pool.tile([64, TO], F32, tag=f"av{h}", name=f"av{h}")
             for h in range(NH)]
    rec12 = [attn_pool.tile([2, 512], F32, tag=f"rc{h}", name=f"rc{h}")
             for h in range(NH)]
    wp_pool = tc.alloc_tile_pool(name="wproj", bufs=1)
    wp = []
    for j in range(NJ):
        t = wp_pool.tile([P, D], BF16, tag=f"wp{j}", name=f"wp{j}")
        nc.sync.dma_start(out=t, in_=wproj_d[j * P:(j + 1) * P, :])
        wp.append(t)
    with (
        tc.tile_pool(name="seps", bufs=3, space="PSUM") as se_ps,
        tc.tile_pool(name="avps", bufs=1, space="PSUM") as av_ps,
        tc.tile_pool(name="sesb", bufs=6) as se_pool,
        tc.tile_pool(name="bcsb", bufs=2) as bc_pool,
    ):
        for hp in range(NJ):
            pt_av_a = av_ps.tile([P, 512], F32, tag="ava", name="ava")
            pt_av_b = av_ps.tile([P, 512], F32, tag="avb", name="avb")
            for tk2 in range(NTK // 2):
                ps_a = se_ps.tile([P, 1024], F32, tag="se", name="psea")
                ps_b = se_ps.tile([P, 1024], F32, tag="se", name="pseb")
                for half in range(2):
                    tk = 2 * tk2 + half
                    ksl = slice(tk * P, (tk + 1) * P)
                    fsl = slice(half * 512, (half + 1) * 512)
                    nc.tensor.matmul(ps_a[:, fsl],
                                     lhsT=k_sb[hp][0:64, ksl],
                                     rhs=q_sb[hp][0:64, :],
                                     start=True, stop=True)
                    nc.tensor.matmul(ps_b[:, fsl],
                                     lhsT=k_sb[hp][64:128, ksl],
                                     rhs=q_sb[hp][64:128, :],
                                     start=True, stop=True)
                se_a = se_pool.tile([P, 1024], BF16, tag="sea", name="sea")
                se_b = se_pool.tile([P, 1024], BF16, tag="seb", name="seb")
                nc.scalar.activation(out=se_a, in_=ps_a, func=AF.Exp)
                nc.scalar.activation(out=se_b, in_=ps_b, func=AF.Exp)
                for half in range(2):
                    tk = 2 * tk2 + half
                    fsl = slice(half * 512, (half + 1) * 512)
                    first = (tk == 0)
                    last = (tk == NTK - 1)
                    nc.tensor.matmul(
                        pt_av_a[:VW, :],
                        lhsT=v_sb[tk][:, (2 * hp) * VW:(2 * hp + 1) * VW],
                        rhs=se_a[:, fsl], start=first, stop=last)
                    nc.tensor.matmul(
                        pt_av_b[:VW, :],
                        lhsT=v_sb[tk][:, (2 * hp + 1) * VW:(2 * hp + 2) * VW],
                        rhs=se_b[:, fsl], start=first, stop=last)
            for head, pt_av in ((0, pt_av_a), (1, pt_av_b)):
                # Evacuate numerator + reciprocal now (DVE is idle during the
                # ACT-bound exp stream); broadcast/divide deferred so no PSUM
                # slot is held across the slow reciprocal.
                h = 2 * hp + head
                nc.vector.tensor_copy(out=av_sb[h][0:64, :],
                                      in_=pt_av[0:64, :])
                nc.vector.reciprocal(out=R(rec12[h]), in_=pt_av[64:66, :])
    # Division tails on evacuated data, fresh PSUM pool.
    with (
        tc.tile_pool(name="divps", bufs=4, space="PSUM") as div_ps,
        tc.tile_pool(name="divsb", bufs=4) as div_pool,
    ):
        for h in range(NH):
            ps_bc = div_ps.tile([64, 512], F32, tag="bc", name="psbc")
            nc.tensor.matmul(ps_bc[:], lhsT=R(half2[:, 0:64]), rhs=R(rec12[h]),
                             start=True, stop=True)
            bc_sb = div_pool.tile([64, 512], F32, tag="bc", name="bcsb")
            nc.vector.tensor_copy(out=bc_sb, in_=ps_bc)
            hp, head = divmod(h, 2)
            nc.vector.tensor_mul(
                out=attn_fm[hp][head * 64:(head + 1) * 64, :],
                in0=av_sb[h][0:64, :], in1=bc_sb)
    v_pool.release()

    # ---------------- Phase 5: proj + residual -> x2 (fp32) ----------------
    x2_pool = tc.alloc_tile_pool(name="x2pool", bufs=1, side="right")
    ctx_pools.append(x2_pool)
    x2_sb = [x2_pool.tile([P, TO], F32, tag=f"x2{j}", name=f"x2{j}")
             for j in range(NJ)]
    wfc1_pool = tc.alloc_tile_pool(name="wfc1", bufs=1, side="right")
    ctx_pools.append(wfc1_pool)
    wf1 = []
    for j in range(NJ):
        t = wfc1_pool.tile([P, DFF], BF16, tag=f"wf1{j}", name=f"wf1{j}")
        nc.sync.dma_start(out=t, in_=wfc1_d[j * P:(j + 1) * P, :])
        wf1.append(t)
    with (
        tc.tile_pool(name="mmps2", bufs=3, space="PSUM") as mm_ps2,
    ):
        for m in range(NJ):
            pt = mm_ps2.tile([P, TO], F32, tag="mm", name="mmproj")
            for j in range(NJ):
                nc.tensor.matmul(pt[:], lhsT=wp[j][:, m * P:(m + 1) * P],
                                 rhs=attn_fm[j],
                                 start=(j == 0), stop=(j == NJ - 1))
            nc.vector.scalar_tensor_tensor(
                out=R(x2_sb[m]), in0=pt, scalar=bproj_sb[:, m:m + 1],
                in1=x_own[m], op0=ALU.add, op1=ALU.add)
    wp_pool.release()
    attn_pool.release()
    persist.release()

    # ---------------- Phase 6: LN2 -> h (bf16) ----------------
    h_pool = tc.alloc_tile_pool(name="hpool", bufs=1, side="right")
    ctx_pools.append(h_pool)
    h_sb = [h_pool.tile([P, TO], BF16, tag=f"h{j}", name=f"h{j}")
            for j in range(NJ)]
    with (
        tc.tile_pool(name="sqpool2", bufs=2) as sq_pool2,
        tc.tile_pool(name="lnps2", bufs=1, space="PSUM") as ln_ps2,
        tc.tile_pool(name="bcps3", bufs=1, space="PSUM") as bc_ps3,
    ):
        pools = (tc, stats, sq_pool2, ln_ps2, bc_ps3)
        _ln_batched(nc, pools, [x2_sb],
                    lambda nt, j: h_sb[j][:, :],
                    ones2, half2, eps2, 1, False)

    # ---------------- Phase 7: fc1 + gelu -> h1 (bf16) ----------------
    h1_pool = tc.alloc_tile_pool(name="h1", bufs=1, side="right")
    ctx_pools.append(h1_pool)
    h1_sb = [h1_pool.tile([P, TO], BF16, tag=f"h1{m}", name=f"h1{m}")
             for m in range(NMLP)]
    with (
        tc.tile_pool(name="mmps3", bufs=4, space="PSUM") as mm_ps3,
    ):
        for m in range(NMLP):
            pt = mm_ps3.tile([P, TO], F32, tag="mm", name="mmfc1")
            for j in range(NJ):
                nc.tensor.matmul(pt[:], lhsT=wf1[j][:, m * P:(m + 1) * P],
                                 rhs=h_sb[j],
                                 start=(j == 0), stop=(j == NJ - 1))
            nc.scalar.activation(out=h1_sb[m], in_=pt, func=AF.Gelu,
                                 bias=bfc1_sb[:, m:m + 1])

    # ---------------- Phase 8: fc2 + residual + store ----------------
    with (
        tc.tile_pool(name="wfc2", bufs=6) as wfc2_pool,
        tc.tile_pool(name="fc2ps", bufs=1, space="PSUM") as fc2_ps,
        tc.tile_pool(name="outsb", bufs=2) as out_pool,
    ):
        pts = [fc2_ps.tile([P, TO], F32, tag=f"fc2_{m}", name=f"fc2_{m}")
               for m in range(NJ)]
        for j in range(NMLP):
            wt = wfc2_pool.tile([P, D], BF16, tag="wf2", name="wf2")
            nc.sync.dma_start(out=wt, in_=wfc2_d[j * P:(j + 1) * P, :])
            for m in range(NJ):
                nc.tensor.matmul(pts[m][:], lhsT=wt[:, m * P:(m + 1) * P],
                                 rhs=h1_sb[j],
                                 start=(j == 0), stop=(j == NMLP - 1))
        for m in range(NJ):
            ot = out_pool.tile([P, TO], F32, tag="out", name="ot")
            nc.vector.scalar_tensor_tensor(
                out=ot, in0=pts[m], scalar=bfc2_sb[:, m:m + 1],
                in1=x2_sb[m], op0=ALU.add, op1=ALU.add)
            nc.sync.dma_start(out=out_fm[m * P:(m + 1) * P, :], in_=ot)

    for pool in reversed(ctx_pools):
        pool.release()


_NC_CACHE = {}


def _get_nc():
    if "nc" not in _NC_CACHE:
        _NC_CACHE["nc"] = _build()
    return _NC_CACHE["nc"]


def _host_prep(inputs):
    f32 = lambda a: np.ascontiguousarray(np.asarray(a, dtype=np.float32))
    x = f32(inputs["x"])            # [2, 2048, 768]
    W_qkv = f32(inputs["W_qkv"])    # [768, 2304]
    b_qkv = f32(inputs["b_qkv"])
    W_proj = f32(inputs["W_proj"])
    b_proj = f32(inputs["b_proj"])
    W_fc1 = f32(inputs["W_fc1"])
    b_fc1 = f32(inputs["b_fc1"])
    W_fc2 = f32(inputs["W_fc2"])
    b_fc2 = f32(inputs["b_fc2"])
    ln1_g = f32(inputs["ln1_g"])
    ln1_b = f32(inputs["ln1_b"])
    ln2_g = f32(inputs["ln2_g"])
    ln2_b = f32(inputs["ln2_b"])

    scale = DH ** -0.5
    wqkv_eff = W_qkv * ln1_g[:, None]
    bqkv_eff = ln1_b @ W_qkv + b_qkv
    wqkv_eff[:, :D] *= scale
    bqkv_eff_q = bqkv_eff[:D] * scale
    bqk = np.concatenate([bqkv_eff_q, bqkv_eff[D:2 * D]]).astype(np.float32)
    bv = bqkv_eff[2 * D:]
    bproj_eff = (b_proj + bv @ W_proj).astype(np.float32)
    wfc1_eff = (W_fc1 * ln2_g[:, None]).astype(np.float32)
    bfc1_eff = (ln2_b @ W_fc1 + b_fc1).astype(np.float32)

    bf = lambda a: np.ascontiguousarray(a.astype(ml_dtypes.bfloat16))
    pack = lambda b: np.ascontiguousarray(
        b.reshape(-1, P).T.astype(np.float32))
    shared = {
        "wqkv": bf(wqkv_eff),
        "bqk": pack(bqk),
        "wproj": bf(W_proj),
        "bproj": pack(bproj_eff),
        "wfc1": bf(wfc1_eff),
        "bfc1": pack(bfc1_eff),
        "wfc2": bf(W_fc2),
        "bfc2": pack(b_fc2),
    }
    in_maps = []
    for c in range(N_CORES):
        b, q = divmod(c, 4)
        xb = np.roll(x[b], -TO * q, axis=0)  # own tokens at rows 0:TO
        m = dict(shared)
        m["x_fm"] = bf(xb.T)
        m["x_own"] = np.ascontiguousarray(xb[:TO].T)
        in_maps.append(m)
    return in_maps


def _run(inputs, trace=False):
    nc = _get_nc()
    in_maps = _host_prep(inputs)
    res = bass_utils.run_bass_kernel_spmd(nc, in_maps, list(range(N_CORES)),
                                          trace=trace)
    B = 2
    out = np.empty((B, TB, D), dtype=np.float32)
    for c in range(N_CORES):
        b, q = divmod(c, 4)
        out[b, TO * q:TO * (q + 1), :] = res.results[c]["out_fm"].T
    return out, res


def kernel(**inputs):
    out, _ = _run(inputs, trace=False)
    return out


if __name__ == "__main__":
    print("building...")
    _get_nc()
    print("built ok")


# revision 21
# speedup vs baseline: 1.0755x; 1.0467x over previous
"""Trainium2 Bass kernel for a dense transformer block (pre-LN, MHA + GELU MLP).

Sharding: 8 cores = 2 batches x 4 sequence-quarters. Each core recomputes
LN1 + K/V for its full batch (zero cross-core communication), and computes
Q/attention/proj/MLP for its own 512 tokens only.

Device works feature-major ([feature, token]); the host pre-transposes x and
post-transposes the output. LN gains/biases are folded into the following
matmul weights on the host; the qk scale (1/8) is folded into W_q; the v bias
is folded into b_proj.

Numerics: matmul operands are bf16 (fp32 PSUM accumulation); the residual
stream (x, x2, out), layernorm statistics, and softmax denominators stay fp32.
LN-statistic / broadcast matmuls run in fp32r.
"""
import sys

sys.path.insert(0, "/opt/trn_rl_repo")

import numpy as np
import ml_dtypes

import concourse.bass as bass  # noqa: F401
import concourse.tile as tile
from concourse import bacc, mybir, bass_utils

F32 = mybir.dt.float32
F32R = mybir.dt.float32r
BF16 = mybir.dt.bfloat16
AF = mybir.ActivationFunctionType
ALU = mybir.AluOpType

P = 128
D = 768
NH = 12
DH = 64
DFF = 3072
TB = 2048      # tokens per batch
TO = 512       # tokens owned per core
NJ = D // P    # 6 feature tiles
NT = TB // TO  # 4 token tiles per batch
NTK = TB // P  # 16 key tiles
NMLP = DFF // P  # 24
EPS = 1e-6
N_CORES = 8
VW = 66        # 64 v cols + 2 ones cols per head


def R(ap):
    return ap.bitcast(F32R)


def _build():
    nc = bacc.Bacc("TRN2", target_bir_lowering=False, debug=False,
                   num_devices=N_CORES)

    x_fm = nc.dram_tensor("x_fm", [D, TB], BF16, kind="ExternalInput").ap()
    x_own_d = nc.dram_tensor("x_own", [D, TO], F32, kind="ExternalInput").ap()
    wqkv = nc.dram_tensor("wqkv", [D, 3 * D], BF16, kind="ExternalInput").ap()
    bqk = nc.dram_tensor("bqk", [P, 12], F32, kind="ExternalInput").ap()
    wproj = nc.dram_tensor("wproj", [D, D], BF16, kind="ExternalInput").ap()
    bproj = nc.dram_tensor("bproj", [P, NJ], F32, kind="ExternalInput").ap()
    wfc1 = nc.dram_tensor("wfc1", [D, DFF], BF16, kind="ExternalInput").ap()
    bfc1 = nc.dram_tensor("bfc1", [P, NMLP], F32, kind="ExternalInput").ap()
    wfc2 = nc.dram_tensor("wfc2", [DFF, D], BF16, kind="ExternalInput").ap()
    bfc2 = nc.dram_tensor("bfc2", [P, NJ], F32, kind="ExternalInput").ap()
    out_fm = nc.dram_tensor("out_fm", [D, TO], F32, kind="ExternalOutput").ap()

    with nc.allow_low_precision(reason="bf16 matmul operands are intentional"), \
            tile.TileContext(nc) as tc:
        _emit(tc, nc, x_fm, x_own_d, wqkv, bqk, wproj, bproj, wfc1, bfc1,
              wfc2, bfc2, out_fm)
    nc.compile()
    return nc


def _ln_batched(nc, pools, x_tiles_by_nt, xn_out_fn, ones_t, half2, eps2,
                n_nt, bf16_in):
    """LayerNorm over n_nt token tiles of 512, one batched stats chain.

    x_tiles_by_nt[nt][j]: input tiles [128, 512] (bf16 if bf16_in else fp32).
    xn_out_fn(nt, j) -> bf16 dest AP [128, 512].
    """
    tc, stats, sq_pool, ln_ps, bc_ps = pools
    ntot = 512 * n_nt
    sum_sb = stats.tile([2, ntot], F32, tag="sum_sb", name="sum_sb")
    sq_sb = stats.tile([2, ntot], F32, tag="sq_sb", name="sq_sb")
    mk = (lambda ap: ap) if bf16_in else R
    sqdt = BF16 if bf16_in else F32
    for nt in range(n_nt):
        x_tiles = x_tiles_by_nt[nt]
        xsq = []
        for j in range(NJ):
            t = sq_pool.tile([P, 512], sqdt, tag=f"xsq{j}", name="xsqt")
            nc.scalar.activation(out=mk(t), in_=x_tiles[j], func=AF.Square)
            xsq.append(t)
        ps_sum = ln_ps.tile([2, 512], F32, tag="lnsum", name="ps_sum")
        ps_sq = ln_ps.tile([2, 512], F32, tag="lnsq", name="ps_sq")
        for j in range(NJ):
            nc.tensor.matmul(ps_sum[:], lhsT=mk(ones_t), rhs=mk(x_tiles[j]),
                             start=(j == 0), stop=(j == NJ - 1))
        for j in range(NJ):
            nc.tensor.matmul(ps_sq[:], lhsT=mk(ones_t), rhs=mk(xsq[j]),
                             start=(j == 0), stop=(j == NJ - 1))
        sl = slice(nt * 512, (nt + 1) * 512)
        nc.vector.tensor_copy(out=sum_sb[:, sl], in_=ps_sum)
        nc.vector.tensor_copy(out=sq_sb[:, sl], in_=ps_sq)
    # var*D^2 = D*sumsq - sum^2 ; rs = exp(-0.5*ln(varD2/D^2 + eps))
    t1 = stats.tile([2, ntot], F32, tag="t1", name="t1")
    nc.vector.scalar_tensor_tensor(out=t1, in0=sum_sb, scalar=-1.0,
                                   in1=sum_sb, op0=ALU.mult, op1=ALU.mult)
    nc.vector.scalar_tensor_tensor(out=t1, in0=sq_sb, scalar=float(D),
                                   in1=t1, op0=ALU.mult, op1=ALU.add)
    nc.scalar.activation(out=t1, in_=t1, func=AF.Ln, bias=eps2,
                         scale=1.0 / (D * D))
    rs = stats.tile([2, ntot], F32, tag="rs", name="rs")
    nc.scalar.activation(out=R(rs), in_=t1, func=AF.Exp, scale=-0.5)
    cc = stats.tile([2, ntot], F32, tag="cc", name="cc")
    nc.vector.scalar_tensor_tensor(out=R(cc), in0=sum_sb, scalar=-1.0 / D,
                                   in1=rs, op0=ALU.mult, op1=ALU.mult)
    for nt in range(n_nt):
        sl = slice(nt * 512, (nt + 1) * 512)
        ps_a = bc_ps.tile([P, 512], F32, tag="bca", name="ps_a")
        nc.tensor.matmul(ps_a[:], lhsT=R(half2), rhs=R(rs[:, sl]),
                         start=True, stop=True)
        ps_c = bc_ps.tile([P, 512], F32, tag="bcc", name="ps_c")
        nc.tensor.matmul(ps_c[:], lhsT=R(half2), rhs=R(cc[:, sl]),
                         start=True, stop=True)
        for j in range(NJ):
            tmp = sq_pool.tile([P, 512], F32, tag=f"tmp{j}", name="xnt")
            nc.vector.tensor_mul(out=tmp, in0=x_tiles_by_nt[nt][j], in1=ps_a)
            nc.vector.tensor_add(out=xn_out_fn(nt, j), in0=tmp, in1=ps_c)


def _emit(tc, nc, x_fm, x_own_d, wqkv, bqk, wproj_d, bproj_d, wfc1_d, bfc1_d,
          wfc2_d, bfc2_d, out_fm):
    ctx_pools = []

    cons_pool = tc.alloc_tile_pool(name="cons", bufs=1)
    ctx_pools.append(cons_pool)
    ones2 = cons_pool.tile([P, 2], F32)
    nc.vector.memset(ones2, 1.0)
    ones2b = cons_pool.tile([P, 2], BF16)
    nc.vector.memset(ones2b, 1.0)
    half2 = cons_pool.tile([2, P], F32)
    nc.vector.memset(half2, 0.5)
    eps2 = cons_pool.tile([2, 1], F32)
    nc.vector.memset(eps2, EPS)
    e0h = cons_pool.tile([2, 64], F32)
    nc.vector.memset(e0h, 0.0)
    nc.vector.memset(e0h[0:1, :], 1.0)
    e1h = cons_pool.tile([2, 64], F32)
    nc.vector.memset(e1h, 1.0)
    nc.vector.memset(e1h[0:1, :], 0.0)

    bqk_sb = cons_pool.tile([P, 12], F32)
    nc.sync.dma_start(out=bqk_sb, in_=bqk)
    bproj_sb = cons_pool.tile([P, NJ], F32)
    nc.sync.dma_start(out=bproj_sb, in_=bproj_d)
    bfc1_sb = cons_pool.tile([P, NMLP], F32)
    nc.sync.dma_start(out=bfc1_sb, in_=bfc1_d)
    bfc2_sb = cons_pool.tile([P, NJ], F32)
    nc.sync.dma_start(out=bfc2_sb, in_=bfc2_d)

    stats = tc.alloc_tile_pool(name="stats", bufs=2)
    ctx_pools.append(stats)

    # k/q bf16; x_own fp32 residual; live until proj.
    persist = tc.alloc_tile_pool(name="persist", bufs=1)
    k_sb = [persist.tile([P, TB], BF16, tag=f"k{j}", name=f"k{j}")
            for j in range(NJ)]
    q_sb = [persist.tile([P, TO], BF16, tag=f"q{j}", name=f"q{j}")
            for j in range(NJ)]
    x_own = [persist.tile([P, TO], F32, tag=f"xo{j}", name=f"xo{j}")
             for j in range(NJ)]

    v_pool = tc.alloc_tile_pool(name="vpool", bufs=1, side="right")
    v_sb = [v_pool.tile([P, NH * VW], BF16, tag=f"v{t}", name=f"v{t}")
            for t in range(NTK)]

    xn_pool = tc.alloc_tile_pool(name="xnpool", bufs=1)
    xn_all = [xn_pool.tile([P, TB], BF16, tag=f"xn{j}", name=f"xn{j}")
              for j in range(NJ)]

    # ---------------- Phase 1: load x (bf16), LN1 -> xn_all (bf16) ---------
    with (
        tc.tile_pool(name="xstream", bufs=1) as xpool,
        tc.tile_pool(name="sqpool", bufs=2) as sq_pool,
        tc.tile_pool(name="lnps", bufs=2, space="PSUM") as ln_ps,
        tc.tile_pool(name="bcps", bufs=2, space="PSUM") as bc_ps,
    ):
        for j in range(NJ):
            nc.sync.dma_start(out=x_own[j], in_=x_own_d[j * P:(j + 1) * P, :])
        pools = (tc, stats, sq_pool, ln_ps, bc_ps)
        for nt in range(NT):
            xt = [xpool.tile([P, TO], BF16, tag=f"xs{nt}_{j}",
                             name=f"xs{nt}_{j}") for j in range(NJ)]
            for j in range(NJ):
                nc.sync.dma_start(
                    out=xt[j],
                    in_=x_fm[j * P:(j + 1) * P, nt * TO:(nt + 1) * TO])
            _ln_batched(nc, pools, [xt],
                        lambda n_, j, nt=nt: xn_all[j][:, nt * TO:(nt + 1) * TO],
                        ones2b, half2, eps2, 1, True)

    # ---------------- Phase 2: Q, V, then K (bf16) ----------------
    # V is emitted before K so that attention (gated on K) starts only after
    # V is resident; av accumulation then never convoys behind the V matmuls.
    with (
        tc.tile_pool(name="wkq", bufs=1) as wkq_pool,
        tc.tile_pool(name="mmps", bufs=4, space="PSUM") as mm_ps,
        tc.tile_pool(name="wv", bufs=1) as wv_pool,
        tc.tile_pool(name="vps5", bufs=2, space="PSUM") as v_ps5,
    ):
        wkq = []
        for j in range(NJ):
            t = wkq_pool.tile([P, 2 * D], BF16, tag=f"wkq{j}", name=f"wkq{j}")
            nc.sync.dma_start(out=t, in_=wqkv[j * P:(j + 1) * P, 0:2 * D])
            wkq.append(t)
        wv = []
        for j in range(NJ):
            t = wv_pool.tile([P, D], BF16, tag=f"wv{j}", name=f"wv{j}")
            nc.sync.dma_start(out=t, in_=wqkv[j * P:(j + 1) * P, 2 * D:3 * D])
            wv.append(t)
        # Q for own tokens
        for m in range(NJ):
            pt = mm_ps.tile([P, TO], F32, tag="mm", name="mmq")
            for j in range(NJ):
                nc.tensor.matmul(pt[:], lhsT=wkq[j][:, m * P:(m + 1) * P],
                                 rhs=xn_all[j][:, 0:TO],
                                 start=(j == 0), stop=(j == NJ - 1))
            nc.vector.tensor_scalar_add(q_sb[m], pt, bqk_sb[:, m:m + 1])
        # K for all tokens
        for m in range(NJ):
            for nt in range(NT):
                pt = mm_ps.tile([P, TO], F32, tag="mm", name="mmk")
                for j in range(NJ):
                    nc.tensor.matmul(
                        pt[:], lhsT=wkq[j][:, D + m * P:D + (m + 1) * P],
                        rhs=xn_all[j][:, nt * TO:(nt + 1) * TO],
                        start=(j == 0), stop=(j == NJ - 1))
                nc.vector.tensor_scalar_add(
                    k_sb[m][:, nt * TO:(nt + 1) * TO], pt,
                    bqk_sb[:, 6 + m:7 + m])
        # V (token-major with ones columns)
        for mt in range(NTK):
            vt = v_sb[mt]
            nc.vector.memset(
                vt.rearrange("p (h w) -> p h w", w=VW)[:, :, 64:66], 1.0)
            pt5 = v_ps5.tile([P, 512], F32, tag="v5", name="v5")
            pt2 = mm_ps.tile([P, TO], F32, tag="mm", name="v2")
            for j in range(NJ):
                lhs = xn_all[j][:, mt * P:(mt + 1) * P]
                nc.tensor.matmul(pt5[:], lhsT=lhs, rhs=wv[j][:, 0:512],
                                 start=(j == 0), stop=(j == NJ - 1))
            for j in range(NJ):
                lhs = xn_all[j][:, mt * P:(mt + 1) * P]
                nc.tensor.matmul(pt2[:, 0:256], lhsT=lhs, rhs=wv[j][:, 512:768],
                                 start=(j == 0), stop=(j == NJ - 1))
            v3 = vt.rearrange("p (h w) -> p h w", w=VW)
            nc.vector.tensor_copy(
                out=v3[:, 0:8, 0:64],
                in_=pt5.rearrange("p (h w) -> p h w", w=64))
            nc.vector.tensor_copy(
                out=v3[:, 8:12, 0:64],
                in_=pt2[:, 0:256].rearrange("p (h w) -> p h w", w=64))
    xn_pool.release()

    # ---------------- Phase 4: attention ----------------
    attn_pool = tc.alloc_tile_pool(name="attnpool", bufs=1)
    attn_fm = [attn_pool.tile([P, TO], BF16, tag=f"at{j}", name=f"at{j}")
               for j in range(NJ)]
    av_sb = [attn_pool.tile([64, TO], F32, tag=f"av{h}", name=f"av{h}")
             for h in range(NH)]
    rec12 = [attn_pool.tile([2, 512], F32, tag=f"rc{h}", name=f"rc{h}")
             for h in range(NH)]
    wp_pool = tc.alloc_tile_pool(name="wproj", bufs=1)
    wp = []
    for j in range(NJ):
        t = wp_pool.tile([P, D], BF16, tag=f"wp{j}", name=f"wp{j}")
        nc.sync.dma_start(out=t, in_=wproj_d[j * P:(j + 1) * P, :])
        wp.append(t)
    with (
        tc.tile_pool(name="seps", bufs=3, space="PSUM") as se_ps,
        tc.tile_pool(name="avps", bufs=1, space="PSUM") as av_ps,
        tc.tile_pool(name="sesb", bufs=6) as se_pool,
        tc.tile_pool(name="bcsb", bufs=2) as bc_pool,
    ):
        for hp in range(NJ):
            pt_av_a = av_ps.tile([P, 512], F32, tag="ava", name="ava")
            pt_av_b = av_ps.tile([P, 512], F32, tag="avb", name="avb")
            for tk2 in range(NTK // 2):
                ps_a = se_ps.tile([P, 1024], F32, tag="se", name="psea")
                ps_b = se_ps.tile([P, 1024], F32, tag="se", name="pseb")
                for half in range(2):
                    tk = 2 * tk2 + half
                    ksl = slice(tk * P, (tk + 1) * P)
                    fsl = slice(half * 512, (half + 1) * 512)
                    nc.tensor.matmul(ps_a[:, fsl],
                                     lhsT=k_sb[hp][0:64, ksl],
                                     rhs=q_sb[hp][0:64, :],
                                     start=True, stop=True)
                    nc.tensor.matmul(ps_b[:, fsl],
                                     lhsT=k_sb[hp][64:128, ksl],
                                     rhs=q_sb[hp][64:128, :],
                                     start=True, stop=True)
                se_a = se_pool.tile([P, 1024], BF16, tag="sea", name="sea")
                se_b = se_pool.tile([P, 1024], BF16, tag="seb", name="seb")
                nc.scalar.activation(out=se_a, in_=ps_a, func=AF.Exp)
                nc.scalar.activation(out=se_b, in_=ps_b, func=AF.Exp)
                for half in range(2):
                    tk = 2 * tk2 + half
                    fsl = slice(half * 512, (half + 1) * 512)
                    first = (tk == 0)
                    last = (tk == NTK - 1)
                    nc.tensor.matmul(
                        pt_av_a[:VW, :],
                        lhsT=v_sb[tk][:, (2 * hp) * VW:(2 * hp + 1) * VW],
                        rhs=se_a[:, fsl], start=first, stop=last)
                    nc.tensor.matmul(
                        pt_av_b[:VW, :],
                        lhsT=v_sb[tk][:, (2 * hp + 1) * VW:(2 * hp + 2) * VW],
                        rhs=se_b[:, fsl], start=first, stop=last)
            for head, pt_av in ((0, pt_av_a), (1, pt_av_b)):
                # Evacuate numerator + reciprocal now (DVE is idle during the
                # ACT-bound exp stream); broadcast/divide deferred so no PSUM
                # slot is held across the slow reciprocal.
                h = 2 * hp + head
                nc.vector.tensor_copy(out=av_sb[h][0:64, :],
                                      in_=pt_av[0:64, :])
                nc.vector.reciprocal(out=R(rec12[h]), in_=pt_av[64:66, :])
    # Division tails on evacuated data, fresh PSUM pool.
    with (
        tc.tile_pool(name="divps", bufs=4, space="PSUM") as div_ps,
        tc.tile_pool(name="divsb", bufs=4) as div_pool,
    ):
        for h in range(NH):
            ps_bc = div_ps.tile([64, 512], F32, tag="bc", name="psbc")
            nc.tensor.matmul(ps_bc[:], lhsT=R(half2[:, 0:64]), rhs=R(rec12[h]),
                             start=True, stop=True)
            bc_sb = div_pool.tile([64, 512], F32, tag="bc", name="bcsb")
            nc.vector.tensor_copy(out=bc_sb, in_=ps_bc)
            hp, head = divmod(h, 2)
            nc.vector.tensor_mul(
                out=attn_fm[hp][head * 64:(head + 1) * 64, :],
                in0=av_sb[h][0:64, :], in1=bc_sb)
    v_pool.release()

    # ---------------- Phase 5: proj + residual -> x2 (fp32) ----------------
    x2_pool = tc.alloc_tile_pool(name="x2pool", bufs=1, side="right")
    ctx_pools.append(x2_pool)
    x2_sb = [x2_pool.tile([P, TO], F32, tag=f"x2{j}", name=f"x2{j}")
             for j in range(NJ)]
    wfc1_pool = tc.alloc_tile_pool(name="wfc1", bufs=1, side="right")
    ctx_pools.append(wfc1_pool)
    wf1 = []
    for j in range(NJ):
        t = wfc1_pool.tile([P, DFF], BF16, tag=f"wf1{j}", name=f"wf1{j}")
        nc.sync.dma_start(out=t, in_=wfc1_d[j * P:(j + 1) * P, :])
        wf1.append(t)
    with (
        tc.tile_pool(name="mmps2", bufs=3, space="PSUM") as mm_ps2,
    ):
        for m in range(NJ):
            pt = mm_ps2.tile([P, TO], F32, tag="mm", name="mmproj")
            for j in range(NJ):
                nc.tensor.matmul(pt[:], lhsT=wp[j][:, m * P:(m + 1) * P],
                                 rhs=attn_fm[j],
                                 start=(j == 0), stop=(j == NJ - 1))
            nc.vector.scalar_tensor_tensor(
                out=R(x2_sb[m]), in0=pt, scalar=bproj_sb[:, m:m + 1],
                in1=x_own[m], op0=ALU.add, op1=ALU.add)
    wp_pool.release()
    attn_pool.release()
    persist.release()

    # ---------------- Phase 6: LN2 -> h (bf16) ----------------
    h_pool = tc.alloc_tile_pool(name="hpool", bufs=1, side="right")
    ctx_pools.append(h_pool)
    h_sb = [h_pool.tile([P, TO], BF16, tag=f"h{j}", name=f"h{j}")
            for j in range(NJ)]
    with (
        tc.tile_pool(name="sqpool2", bufs=2) as sq_pool2,
        tc.tile_pool(name="lnps2", bufs=1, space="PSUM") as ln_ps2,
        tc.tile_pool(name="bcps3", bufs=1, space="PSUM") as bc_ps3,
    ):
        pools = (tc, stats, sq_pool2, ln_ps2, bc_ps3)
        _ln_batched(nc, pools, [x2_sb],
                    lambda nt, j: h_sb[j][:, :],
                    ones2, half2, eps2, 1, False)

    # ---------------- Phase 7: fc1 + gelu -> h1 (bf16) ----------------
    h1_pool = tc.alloc_tile_pool(name="h1", bufs=1, side="right")
    ctx_pools.append(h1_pool)
    h1_sb = [h1_pool.tile([P, TO], BF16, tag=f"h1{m}", name=f"h1{m}")
             for m in range(NMLP)]
    with (
        tc.tile_pool(name="mmps3", bufs=4, space="PSUM") as mm_ps3,
    ):
        for m in range(NMLP):
            pt = mm_ps3.tile([P, TO], F32, tag="mm", name="mmfc1")
            for j in range(NJ):
                nc.tensor.matmul(pt[:], lhsT=wf1[j][:, m * P:(m + 1) * P],
                                 rhs=h_sb[j],
                                 start=(j == 0), stop=(j == NJ - 1))
            nc.scalar.activation(out=h1_sb[m], in_=pt, func=AF.Gelu,
                                 bias=bfc1_sb[:, m:m + 1])

    # ---------------- Phase 8: fc2 + residual + store ----------------
    with (
        tc.tile_pool(name="wfc2", bufs=6) as wfc2_pool,
        tc.tile_pool(name="fc2ps", bufs=1, space="PSUM") as fc2_ps,
        tc.tile_pool(name="outsb", bufs=2) as out_pool,
    ):
        pts = [fc2_ps.tile([P, TO], F32, tag=f"fc2_{m}", name=f"fc2_{m}")
               for m in range(NJ)]
        for j in range(NMLP):
            wt = wfc2_pool.tile([P, D], BF16, tag="wf2", name="wf2")
            nc.sync.dma_start(out=wt, in_=wfc2_d[j * P:(j + 1) * P, :])
            for m in range(NJ):
                nc.tensor.matmul(pts[m][:], lhsT=wt[:, m * P:(m + 1) * P],
                                 rhs=h1_sb[j],
                                 start=(j == 0), stop=(j == NMLP - 1))
        for m in range(NJ):
            ot = out_pool.tile([P, TO], F32, tag="out", name="ot")
            nc.vector.scalar_tensor_tensor(
                out=ot, in0=pts[m], scalar=bfc2_sb[:, m:m + 1],
                in1=x2_sb[m], op0=ALU.add, op1=ALU.add)
            nc.sync.dma_start(out=out_fm[m * P:(m + 1) * P, :], in_=ot)

    for pool in reversed(ctx_pools):
        pool.release()


_NC_CACHE = {}


def _get_nc():
    if "nc" not in _NC_CACHE:
        _NC_CACHE["nc"] = _build()
    return _NC_CACHE["nc"]


def _host_prep(inputs):
    f32 = lambda a: np.ascontiguousarray(np.asarray(a, dtype=np.float32))
    x = f32(inputs["x"])            # [2, 2048, 768]
    W_qkv = f32(inputs["W_qkv"])    # [768, 2304]
    b_qkv = f32(inputs["b_qkv"])
    W_proj = f32(inputs["W_proj"])
    b_proj = f32(inputs["b_proj"])
    W_fc1 = f32(inputs["W_fc1"])
    b_fc1 = f32(inputs["b_fc1"])
    W_fc2 = f32(inputs["W_fc2"])
    b_fc2 = f32(inputs["b_fc2"])
    ln1_g = f32(inputs["ln1_g"])
    ln1_b = f32(inputs["ln1_b"])
    ln2_g = f32(inputs["ln2_g"])
    ln2_b = f32(inputs["ln2_b"])

    scale = DH ** -0.5
    wqkv_eff = W_qkv * ln1_g[:, None]
    bqkv_eff = ln1_b @ W_qkv + b_qkv
    wqkv_eff[:, :D] *= scale
    bqkv_eff_q = bqkv_eff[:D] * scale
    bqk = np.concatenate([bqkv_eff_q, bqkv_eff[D:2 * D]]).astype(np.float32)
    bv = bqkv_eff[2 * D:]
    bproj_eff = (b_proj + bv @ W_proj).astype(np.float32)
    wfc1_eff = (W_fc1 * ln2_g[:, None]).astype(np.float32)
    bfc1_eff = (ln2_b @ W_fc1 + b_fc1).astype(np.float32)

    bf = lambda a: np.ascontiguousarray(a.astype(ml_dtypes.bfloat16))
    pack = lambda b: np.ascontiguousarray(
        b.reshape(-1, P).T.astype(np.float32))
    shared = {
        "wqkv": bf(wqkv_eff),
        "bqk": pack(bqk),
        "wproj": bf(W_proj),
        "bproj": pack(bproj_eff),
        "wfc1": bf(wfc1_eff),
        "bfc1": pack(bfc1_eff),
        "wfc2": bf(W_fc2),
        "bfc2": pack(b_fc2),
    }
    in_maps = []
    for c in range(N_CORES):
        b, q = divmod(c, 4)
        xb = np.roll(x[b], -TO * q, axis=0)  # own tokens at rows 0:TO
        m = dict(shared)
        m["x_fm"] = bf(xb.T)
        m["x_own"] = np.ascontiguousarray(xb[:TO].T)
        in_maps.append(m)
    return in_maps


def _run(inputs, trace=False):
    nc = _get_nc()
    in_maps = _host_prep(inputs)
    res = bass_utils.run_bass_kernel_spmd(nc, in_maps, list(range(N_CORES)),
                                          trace=trace)
    B = 2
    out = np.empty((B, TB, D), dtype=np.float32)
    for c in range(N_CORES):
        b, q = divmod(c, 4)
        out[b, TO * q:TO * (q + 1), :] = res.results[c]["out_fm"].T
    return out, res


def kernel(**inputs):
    out, _ = _run(inputs, trace=False)
    return out


if __name__ == "__main__":
    print("building...")
    _get_nc()
    print("built ok")
